# revision 20
# baseline (speedup 1.0000x reference)
"""Trainium2 Bass kernel for BilingualSentenceClassifier (segment_reduce).

Computes, for B=64 samples of S=2048 tokens with D=1024 embedding dims:
  sent1 = mean(embs[1:idx1])            (idx1 = first EOS position)
  sent2 = mean(embs[idx1+2:idx2])       (idx2 = first PAD position - 1)
  logits = tanh(concat(sent1, sent2) @ dense_w + dense_b) @ out_w + out_b

Strategy: pure data parallel over 8 NeuronCores (8 samples per core).
The kernel is HBM-bandwidth bound, so the embedding stream is cut to the
minimum: only tokens that carry nonzero mask weight are shipped, packed
back-to-back at token granularity (samples balanced across cores by exact
token count), and quantized to fp8 e3m4 (the segment means + dense head
keep ~9e-3 relative error, well under the 2e-2 gate).  dense_w streams in
fp8 e3m4 too, pre-scaled by an exact power of two that the tanh
activation's scale input undoes.  Mask weights (1/n at member tokens)
ride in fp16 as the matmul moving operand, so the per-sample
normalization is exact to fp16.

Phase 1 uses the embedding chunk as the *stationary* operand ([128 tok,
128 dims] slices) against the [128 tok, 16] weight matrix, producing the
segment means directly transposed ([dim, 2*sample]) in a single PSUM
bank, which feeds the dense head with no transpose stage: the head runs
with dense_w blocks stationary and [128, 8] moving slices, dense_w
streaming *after* the embeddings so the head chases the tail of the DMA
stream.  Everything downstream of the segment sums stays in fp16/fp32.
"""

import sys

sys.path.insert(0, "/opt/trn_rl_repo")

import numpy as np
import ml_dtypes

import concourse.bass as bass
import concourse.tile as tile
from concourse import mybir
import bass_rust
from concourse.bass_utils import run_bass_kernel_spmd

B, S, D = 64, 2048, 1024
EOS_ID, PAD_ID = 2, 1
N_CORES = 8
B_LOC = B // N_CORES          # samples per core
KD = 16                       # 128-row contraction blocks in dense_w
KH = D // 128                 # 128-row contraction blocks in out_w
G = 16                        # sequence chunks per embedding DMA
ALPHA = 128.0                 # dense_w fp8 pre-scale (exact power of two)

F32 = mybir.dt.float32
F32R = mybir.dt.float32r
BF16 = mybir.dt.bfloat16
F16 = mybir.dt.float16
F8E3 = mybir.dt.float8e3

NP_E3M4 = ml_dtypes.float8_e3m4
NP_BF16 = ml_dtypes.bfloat16


def _split_excess_waits(nc, max_waits=1):
    """This container's walrus rejects instructions carrying more than 1-2
    sync waits (e.g. the Tile tail drain).  Hoist excess waits onto
    preceding same-engine NOPs — semantically identical: the engine's
    sequencer blocks on the NOP's wait before dispatching the original
    instruction."""
    cnt = 0
    for f in nc.m.functions:
        for blk in f.blocks:
            out = []
            changed = False
            for inst in blk.instructions:
                si = inst.sync_info
                if si is not None and len(si.on_wait) > max_waits:
                    waits = list(si.on_wait)
                    for w in waits[:-max_waits]:
                        cnt += 1
                        nop = mybir.InstNoOp(name=f"{inst.name}-hw{cnt}")
                        nop.engine = inst.engine
                        nop.sync_info = bass_rust.SyncInfo(on_wait=[w], on_update=[])
                        out.append(nop)
                    inst.sync_info = bass_rust.SyncInfo(
                        on_wait=waits[-max_waits:], on_update=list(si.on_update)
                    )
                    changed = True
                out.append(inst)
            if changed:
                blk.instructions = out
    return cnt


def _build_program_untrimmed(T, rows_last):
    nc = _build_program(T, rows_last, trim=False)
    return nc


def _build_program(T, rows_last, trim=True):
    """SPMD program processing T 128-token chunks of packed embeddings; the
    final chunk only carries rows_last valid token rows."""
    nc = bass.Bass("TRN2", target_bir_lowering=False, debug=False, num_devices=N_CORES)

    embs = nc.dram_tensor("embs", [T * 128, D], F8E3, kind="ExternalInput")
    wm = nc.dram_tensor("wm", [128, T * 16], F16, kind="ExternalInput")
    dw = nc.dram_tensor("dw", [2 * D, D], F8E3, kind="ExternalInput")
    db = nc.dram_tensor("db", [1, D], F32R, kind="ExternalInput")
    # ow pre-packed on host to [128, KH*2] (partition-major) so the DMA
    # moves one 32B run per partition instead of 2048 4-byte scatters
    ow = nc.dram_tensor("ow", [128, KH * 2], BF16, kind="ExternalInput")
    ob = nc.dram_tensor("ob", [1, 2], F32R, kind="ExternalInput")
    ones = nc.dram_tensor("ones", [1, B_LOC], F32R, kind="ExternalInput")
    out = nc.dram_tensor("out", [2, B_LOC], F32, kind="ExternalOutput")

    groups = []
    t0 = 0
    while t0 < T:
        groups.append((t0, min(G, T - t0)))
        t0 += min(G, T - t0)

    with tile.TileContext(nc) as tc:
        with (
            tc.tile_pool(name="sb", bufs=1) as consts,
            tc.tile_pool(name="ps", bufs=1, space="PSUM") as ps,
        ):
            embp = dwp = consts
            # ---- phase 1: segment sums, directly transposed ---------------
            # xt_ps[p, s, q] = sum_tok emb[tok, 128 s + p] * wm[tok, q]
            # (q = 2 j + r selects sample j / segment r; wm carries 1/n).
            # All 8 dim-slices accumulate into one PSUM bank: start=True only
            # on the very first matmul (clears the bank's has_written bits);
            # every later first-touch overwrites-where-unset, then
            # accumulates.
            # The first embedding group's DMA is issued before the params so
            # the param DMAs' issue overhead hides under its transfer.
            xt_ps = ps.tile([128, 8, 16], F32, tag="xt_ps")
            wm_t = consts.tile([128, T, 16], F16, tag="wm")
            ow_t = consts.tile([128, KH, 2], BF16, tag="ow")
            db_t = consts.tile([1, D], F32R, tag="db")
            ob_t = consts.tile([1, 2], F32R, tag="ob")
            ones_t = consts.tile([1, B_LOC], F32R, tag="ones")
            warm = consts.tile([1, 8], F32, tag="warm")
            for g, (gt, gn) in enumerate(groups):
                et = embp.tile([128, gn, D], F8E3, tag=f"emb{g}")
                nfull = gn if (gt + gn < T or rows_last == 128) else gn - 1
                if nfull:
                    src = embs.ap()[gt * 128 : (gt + nfull) * 128, :]
                    nc.sync.dma_start(
                        out=et[:, :nfull, :],
                        in_=src.rearrange("(n p) d -> p n d", p=128),
                    )
                if nfull < gn:
                    base = (gt + nfull) * 128
                    nc.sync.dma_start(
                        out=et[:rows_last, nfull, :],
                        in_=embs.ap()[base : base + rows_last, :],
                    )
                if g == 0:
                    nc.sync.dma_start(out=wm_t[:], in_=wm.ap())
                    nc.sync.dma_start(out=ow_t[:], in_=ow.ap())
                    nc.sync.dma_start(out=db_t[:], in_=db.ap())
                    nc.sync.dma_start(out=ob_t[:], in_=ob.ap())
                    nc.sync.dma_start(out=ones_t[:], in_=ones.ap())
                    # warm the ScalarE Tanh LUT while the stream runs
                    nc.vector.memset(warm[:], 0.0)
                    nc.scalar.activation(
                        warm[:], warm[:], mybir.ActivationFunctionType.Tanh
                    )
                for c in range(gn):
                    t = gt + c
                    rows = 128 if t < T - 1 else rows_last
                    for s in range(8):
                        nc.tensor.matmul(
                            xt_ps[:, s, :],
                            et[0:rows, c, s * 128 : s * 128 + 128],
                            wm_t[0:rows, t, :],
                            start=(t == 0 and s == 0),
                            stop=(t == T - 1),
                        )
            xt = consts.tile([128, 8, 16], F16, tag="xt")
            nc.vector.tensor_copy(xt[:], xt_ps[:])

            # dense_w streams after the embeddings; the head chases it.
            # Batched into >=3-block DMAs (transfer > the 625ns issue cost)
            # with a small final DMA so the post-stream matmul tail is short.
            dw_t = dwp.tile([128, KD, D], F8E3, tag="dw")
            k0 = 0
            for nblk in (3, 3, 3, 3, 3, 1):
                src = dw.ap()[128 * k0 : 128 * (k0 + nblk), :]
                nc.sync.dma_start(
                    out=dw_t[:, k0 : k0 + nblk, :],
                    in_=src.rearrange("(n p) d -> p n d", p=128),
                )
                k0 += nblk

            # ---- phase 2: hidden^T = tanh(dense_w^T x + db), k-major ------
            # The db bias matmuls lead the group (start=True on the first
            # clears the bank) so nothing but the final k-block's 8 matmuls
            # remains after the last dense_w DMA lands.
            ph = ps.tile([128, KH, B_LOC], F32, tag="ph")
            for h in range(KH):
                nc.tensor.matmul(
                    ph[:, h, :],
                    db_t[0:1, h * 128 : h * 128 + 128],
                    ones_t[0:1, :],
                    start=(h == 0),
                    stop=False,
                )
            for k in range(KD):
                r, s = divmod(k, 8)
                mov = xt[:, s, r::2]
                for h in range(KH):
                    nc.tensor.matmul(
                        ph[:, h, :],
                        dw_t[:, k, h * 128 : h * 128 + 128],
                        mov,
                        start=False,
                        stop=(k == KD - 1),
                    )
            # ph holds ALPHA*(x @ dense_w + db); the activation's exact
            # power-of-two scale undoes the fp8 weight pre-scale
            ht = consts.tile([128, KH, B_LOC], F16, tag="ht")
            nc.scalar.activation(
                ht[:], ph[:], mybir.ActivationFunctionType.Tanh, scale=1.0 / ALPHA
            )

            # ---- phase 3: logits^T = out_w^T h + ob -----------------------
            pl = ps.tile([2, B_LOC], F32, tag="pl")
            for h in range(KH):
                nc.tensor.matmul(
                    pl[:], ow_t[:, h, :], ht[:, h, :], start=(h == 0), stop=False
                )
            nc.tensor.matmul(
                pl[:], ob_t[0:1, :], ones_t[0:1, :], start=False, stop=True
            )
            lg = consts.tile([2, B_LOC], F32, tag="lg")
            nc.vector.tensor_copy(lg[:], pl[:])
            nc.sync.dma_start(out=out.ap(), in_=lg[:])

    _split_excess_waits(nc)
    if trim:
        try:
            _trim_framework_sync(nc, do_prologue=True, do_epilogue=True)
        except Exception:
            # the trim is a pure optimization; an unexpected program shape
            # must not break the build (rebuild untrimmed)
            return _build_program(T, rows_last, trim=False)
    return nc


def _trim_framework_sync(nc, do_prologue=True, do_epilogue=True):
    """Post-pass on the Tile-generated program:
    1. Remove the prologue all-engine barrier — every body dependency is
       already semaphore-protected, and each engine's own setup precedes
       its body in program order.
    2. Drop the epilogue drain's waits on DMA-lane semaphores that were
       already consumed by in-body readers; only the final (output) DMA's
       lane has no in-body consumer, so only its wait is load-bearing.
    The epilogue barrier + semaphore clears are kept so repeat launches
    still start from clean semaphore state."""
    f = nc.m.functions[0]
    pre, body, epi = f.blocks[0], f.blocks[1], f.blocks[-1]

    removed_update_sems = set()
    out = []
    for inst in pre.instructions if do_prologue else []:
        if isinstance(inst, mybir.InstEventSemaphore) and inst.name.startswith(
            "barrier_"
        ):
            if inst.sync_info:
                removed_update_sems.update(u.id for u in inst.sync_info.on_update)
            continue
        if isinstance(inst, mybir.InstDrain) and inst.sync_info:
            # strip the barrier's waits AND its sem increments together;
            # leaving the increments corrupts the epilogue barrier counts
            inst.sync_info = None
        out.append(inst)
    if do_prologue:
        pre.instructions = out

    # epilogue: rebuild it minimally.  The only load-bearing wait is the
    # output DMA's lane semaphore (every other DMA's semaphore was already
    # consumed by an in-body reader, and the out DMA transitively follows
    # all compute), so the epilogue becomes: SP drain waiting that
    # semaphore, the semaphore RANGE_CLEAR on SP right after it (clean
    # state for repeat launches), and one sync-free pipeline drain per
    # other engine.  Both Tile all-engine barrier rounds are dead weight.
    if do_epilogue:
        dmas = [i for i in body.instructions if isinstance(i, mybir.InstDMACopy)]
        out_upd = dmas[-1].sync_info.on_update[0]
        out_sem = out_upd.id
        final_val = sum(
            int(u.update_value or 0)
            for d in dmas
            for u in (d.sync_info.on_update if d.sync_info else [])
            if u.id == out_sem
        )
        out_wait = None
        for inst in epi.instructions:
            for w in inst.sync_info.on_wait if inst.sync_info else []:
                if w.id == out_sem:
                    out_wait = w
        assert out_wait is not None and int(out_wait.wait_value) == final_val, (
            f"epilogue lacks a wait for the out DMA sem {out_sem} at {final_val}"
        )
        sp_drain = None
        clear_inst = None
        eng_drains = {}
        for inst in epi.instructions:
            tn = type(inst).__name__
            if tn == "InstISA" and clear_inst is None:
                clear_inst = inst
            if tn == "InstDrain":
                key = str(inst.engine)
                if "SP" in key and sp_drain is None:
                    sp_drain = inst
                elif "SP" not in key:
                    eng_drains.setdefault(key, inst)
        assert sp_drain is not None and clear_inst is not None
        sp_drain.sync_info = bass_rust.SyncInfo(on_wait=[out_wait], on_update=[])
        clear_inst.engine = sp_drain.engine
        rebuilt = [sp_drain, clear_inst]
        for inst in eng_drains.values():
            inst.sync_info = None
            rebuilt.append(inst)
        epi.instructions = rebuilt

    # sanity: every waited (sem, value) must be coverable by total updates,
    # and nothing may wait on a semaphore whose barrier update was removed
    from collections import defaultdict

    updates = defaultdict(int)
    waited = defaultdict(int)
    for blk in f.blocks:
        for inst in blk.instructions:
            si = inst.sync_info
            if not si:
                continue
            for u in si.on_update:
                updates[u.id] += int(u.update_value or 0)
            for w in si.on_wait:
                if w.wait_value is not None and "ge" in str(w.wait_mode):
                    waited[w.id] = max(waited[w.id], int(w.wait_value))
    for sem_id, val in waited.items():
        assert updates[sem_id] >= val, (
            f"sem {sem_id}: waits up to {val} but only {updates[sem_id]} updates"
        )
        assert sem_id not in removed_update_sems or updates[sem_id] >= val


_PROGRAM_CACHE = {}
LAST_RESULTS = None


def kernel(embs, input_ids, dense_w, dense_b, out_w, out_b):
    embs = np.ascontiguousarray(np.asarray(embs, dtype=np.float32))
    ids = np.asarray(input_ids)
    dense_w = np.asarray(dense_w, dtype=np.float32)
    dense_b = np.asarray(dense_b, dtype=np.float32)
    out_w = np.asarray(out_w, dtype=np.float32)
    out_b = np.asarray(out_b, dtype=np.float32)

    # host-side mask metadata — exactly the reference's argmax semantics
    idx1 = np.argmax(ids == EOS_ID, axis=-1)
    idx2 = np.argmax(ids == PAD_ID, axis=-1) - 1
    pos = np.arange(S)
    m1 = ((pos >= 1) & (pos < idx1[:, None])).astype(np.float32)
    m2 = ((pos >= idx1[:, None] + 2) & (pos < idx2[:, None])).astype(np.float32)
    n1 = m1.sum(-1, keepdims=True)
    n2 = m2.sum(-1, keepdims=True)
    # empty segments give 0/0 = NaN in the reference; keep device weights
    # finite (zero) and reinstate the NaN on the host afterwards
    w1 = np.where(n1 > 0, m1 / np.maximum(n1, 1), 0.0).astype(np.float32)
    w2 = np.where(n2 > 0, m2 / np.maximum(n2, 1), 0.0).astype(np.float32)
    nan_rows = (n1[:, 0] == 0) | (n2[:, 0] == 0)

    used = (m1 + m2) > 0                      # [B, S] tokens with weight
    tokens = used.sum(axis=1).astype(int)

    # balance samples across cores by exact token count (8 per core)
    order = np.argsort(-tokens, kind="stable")
    loads = np.zeros(N_CORES, dtype=int)
    counts = np.zeros(N_CORES, dtype=int)
    assign = [[] for _ in range(N_CORES)]
    for b in order:
        free = np.nonzero(counts < B_LOC)[0]
        core = free[np.argmin(loads[free])]
        assign[core].append(int(b))
        loads[core] += tokens[b]
        counts[core] += 1
    T = max(1, int(-(-loads.max() // 128)))
    rows_last = max(1, int(loads.max() - (T - 1) * 128))

    key = (T, rows_last)
    if key not in _PROGRAM_CACHE:
        _PROGRAM_CACHE[key] = _build_program(T, rows_last)
    nc = _PROGRAM_CACHE[key]

    dw_b = (dense_w * ALPHA).astype(NP_E3M4)
    ow_b = np.ascontiguousarray(
        out_w.reshape(KH, 128, 2).transpose(1, 0, 2).reshape(128, KH * 2)
    ).astype(NP_BF16)
    db_r = (dense_b * ALPHA).reshape(1, D)
    ob_r = out_b.reshape(1, 2)
    ones = np.ones((1, B_LOC), np.float32)

    in_maps = []
    for core in range(N_CORES):
        packed = np.zeros((T * 128, D), dtype=NP_E3M4)
        wmf = np.zeros((T * 128, 16), dtype=np.float32)
        off = 0
        for j, b in enumerate(assign[core]):
            posb = np.nonzero(used[b])[0]
            L = len(posb)
            if L:
                packed[off : off + L] = embs[b, posb].astype(NP_E3M4)
                wmf[off : off + L, 2 * j] = w1[b, posb]
                wmf[off : off + L, 2 * j + 1] = w2[b, posb]
            off += L
        wm16 = np.ascontiguousarray(
            wmf.reshape(T, 128, 16).transpose(1, 0, 2).reshape(128, T * 16)
        ).astype(np.float16)
        in_maps.append(
            {
                "embs": packed,
                "wm": wm16,
                "dw": dw_b,
                "db": db_r,
                "ow": ow_b,
                "ob": ob_r,
                "ones": ones,
            }
        )

    res = run_bass_kernel_spmd(nc, in_maps, list(range(N_CORES)))
    global LAST_RESULTS
    LAST_RESULTS = res

    logits = np.empty((B, 2), np.float32)
    for core in range(N_CORES):
        logits[assign[core]] = res.results[core]["out"].T
    logits[nan_rows] = np.nan
    return logits


# revision 21
# speedup vs baseline: 1.0173x; 1.0173x over previous
"""Trainium2 Bass kernel for BilingualSentenceClassifier (segment_reduce).

Computes, for B=64 samples of S=2048 tokens with D=1024 embedding dims:
  sent1 = mean(embs[1:idx1])            (idx1 = first EOS position)
  sent2 = mean(embs[idx1+2:idx2])       (idx2 = first PAD position - 1)
  logits = tanh(concat(sent1, sent2) @ dense_w + dense_b) @ out_w + out_b

Strategy: pure data parallel over 8 NeuronCores (8 samples per core).
The kernel is HBM-bandwidth bound, so the embedding stream is cut to the
minimum: only tokens that carry nonzero mask weight are shipped, packed
back-to-back at token granularity (samples balanced across cores by exact
token count), and quantized to fp8 e3m4 (the segment means + dense head
keep ~9e-3 relative error, well under the 2e-2 gate).  dense_w streams in
fp8 e3m4 too, pre-scaled by an exact power of two that the tanh
activation's scale input undoes.  Mask weights (1/n at member tokens)
ride in fp16 as the matmul moving operand, so the per-sample
normalization is exact to fp16.

Phase 1 uses the embedding chunk as the *stationary* operand ([128 tok,
128 dims] slices) against the [128 tok, 16] weight matrix, producing the
segment means directly transposed ([dim, 2*sample]) in a single PSUM
bank, which feeds the dense head with no transpose stage: the head runs
with dense_w blocks stationary and [128, 8] moving slices, dense_w
streaming *after* the embeddings so the head chases the tail of the DMA
stream.  Everything downstream of the segment sums stays in fp16/fp32.
"""

import sys

sys.path.insert(0, "/opt/trn_rl_repo")

import numpy as np
import ml_dtypes

import concourse.bass as bass
import concourse.tile as tile
from concourse import mybir
import bass_rust
from concourse.bass_utils import run_bass_kernel_spmd

B, S, D = 64, 2048, 1024
EOS_ID, PAD_ID = 2, 1
N_CORES = 8
B_LOC = B // N_CORES          # samples per core
KD = 16                       # 128-row contraction blocks in dense_w
KH = D // 128                 # 128-row contraction blocks in out_w
G = 16                        # sequence chunks per embedding DMA
ALPHA = 128.0                 # dense_w fp8 pre-scale (exact power of two)

F32 = mybir.dt.float32
F32R = mybir.dt.float32r
BF16 = mybir.dt.bfloat16
F16 = mybir.dt.float16
F8E3 = mybir.dt.float8e3

NP_E3M4 = ml_dtypes.float8_e3m4
NP_BF16 = ml_dtypes.bfloat16


def _split_excess_waits(nc, max_waits=1):
    """This container's walrus rejects instructions carrying more than 1-2
    sync waits (e.g. the Tile tail drain).  Hoist excess waits onto
    preceding same-engine NOPs — semantically identical: the engine's
    sequencer blocks on the NOP's wait before dispatching the original
    instruction."""
    cnt = 0
    for f in nc.m.functions:
        for blk in f.blocks:
            out = []
            changed = False
            for inst in blk.instructions:
                si = inst.sync_info
                if si is not None and len(si.on_wait) > max_waits:
                    waits = list(si.on_wait)
                    for w in waits[:-max_waits]:
                        cnt += 1
                        nop = mybir.InstNoOp(name=f"{inst.name}-hw{cnt}")
                        nop.engine = inst.engine
                        nop.sync_info = bass_rust.SyncInfo(on_wait=[w], on_update=[])
                        out.append(nop)
                    inst.sync_info = bass_rust.SyncInfo(
                        on_wait=waits[-max_waits:], on_update=list(si.on_update)
                    )
                    changed = True
                out.append(inst)
            if changed:
                blk.instructions = out
    return cnt


def _build_program_untrimmed(T, rows_last):
    nc = _build_program(T, rows_last, trim=False)
    return nc


def _build_program(T, rows_last, trim=True):
    """SPMD program processing T 128-token chunks of packed embeddings; the
    final chunk only carries rows_last valid token rows."""
    nc = bass.Bass("TRN2", target_bir_lowering=False, debug=False, num_devices=N_CORES)

    embs = nc.dram_tensor("embs", [T * 128, D], F8E3, kind="ExternalInput")
    wm = nc.dram_tensor("wm", [128, T * 16], F16, kind="ExternalInput")
    dw = nc.dram_tensor("dw", [2 * D, D], F8E3, kind="ExternalInput")
    db = nc.dram_tensor("db", [1, D], F32R, kind="ExternalInput")
    # ow pre-packed on host to [128, KH*2] (partition-major) so the DMA
    # moves one 32B run per partition instead of 2048 4-byte scatters
    ow = nc.dram_tensor("ow", [128, KH * 2], BF16, kind="ExternalInput")
    ob = nc.dram_tensor("ob", [1, 2], F32R, kind="ExternalInput")
    ones = nc.dram_tensor("ones", [1, B_LOC], F32R, kind="ExternalInput")
    out = nc.dram_tensor("out", [2, B_LOC], F32, kind="ExternalOutput")

    groups = []
    t0 = 0
    while t0 < T:
        groups.append((t0, min(G, T - t0)))
        t0 += min(G, T - t0)

    with tile.TileContext(nc) as tc:
        with (
            tc.tile_pool(name="sb", bufs=1) as consts,
            tc.tile_pool(name="ps", bufs=1, space="PSUM") as ps,
        ):
            embp = dwp = consts
            # ---- phase 1: segment sums, directly transposed ---------------
            # xt_ps[p, s, q] = sum_tok emb[tok, 128 s + p] * wm[tok, q]
            # (q = 2 j + r selects sample j / segment r; wm carries 1/n).
            # All 8 dim-slices accumulate into one PSUM bank: start=True only
            # on the very first matmul (clears the bank's has_written bits);
            # every later first-touch overwrites-where-unset, then
            # accumulates.
            # The first embedding group's DMA is issued before the params so
            # the param DMAs' issue overhead hides under its transfer.
            xt_ps = ps.tile([128, 8, 16], F32, tag="xt_ps")
            wm_t = consts.tile([128, T, 16], F16, tag="wm")
            ow_t = consts.tile([128, KH, 2], BF16, tag="ow")
            db_t = consts.tile([1, D], F32R, tag="db")
            ob_t = consts.tile([1, 2], F32R, tag="ob")
            ones_t = consts.tile([1, B_LOC], F32R, tag="ones")
            warm = consts.tile([1, 8], F32, tag="warm")
            for g, (gt, gn) in enumerate(groups):
                et = embp.tile([128, gn, D], F8E3, tag=f"emb{g}")
                nfull = gn if (gt + gn < T or rows_last == 128) else gn - 1
                if nfull:
                    src = embs.ap()[gt * 128 : (gt + nfull) * 128, :]
                    nc.sync.dma_start(
                        out=et[:, :nfull, :],
                        in_=src.rearrange("(n p) d -> p n d", p=128),
                    )
                if nfull < gn:
                    base = (gt + nfull) * 128
                    nc.sync.dma_start(
                        out=et[:rows_last, nfull, :],
                        in_=embs.ap()[base : base + rows_last, :],
                    )
                if g == 0:
                    nc.sync.dma_start(out=wm_t[:], in_=wm.ap())
                    nc.sync.dma_start(out=ow_t[:], in_=ow.ap())
                    nc.sync.dma_start(out=db_t[:], in_=db.ap())
                    nc.sync.dma_start(out=ob_t[:], in_=ob.ap())
                    nc.sync.dma_start(out=ones_t[:], in_=ones.ap())
                    # warm the ScalarE Tanh LUT while the stream runs
                    nc.vector.memset(warm[:], 0.0)
                    nc.scalar.activation(
                        warm[:], warm[:], mybir.ActivationFunctionType.Tanh
                    )
                for c in range(gn):
                    t = gt + c
                    rows = 128 if t < T - 1 else rows_last
                    for s in range(8):
                        nc.tensor.matmul(
                            xt_ps[:, s, :],
                            et[0:rows, c, s * 128 : s * 128 + 128],
                            wm_t[0:rows, t, :],
                            start=(t == 0 and s == 0),
                            stop=(t == T - 1),
                        )
            xt = consts.tile([128, 8, 16], F16, tag="xt")
            nc.vector.tensor_copy(xt[:], xt_ps[:])

            # dense_w streams after the embeddings; the head chases it.
            # Batched into >=3-block DMAs (transfer > the 625ns issue cost)
            # with a small final DMA so the post-stream matmul tail is short.
            dw_t = dwp.tile([128, KD, D], F8E3, tag="dw")
            k0 = 0
            for nblk in (3, 3, 3, 3, 3, 1):
                src = dw.ap()[128 * k0 : 128 * (k0 + nblk), :]
                nc.sync.dma_start(
                    out=dw_t[:, k0 : k0 + nblk, :],
                    in_=src.rearrange("(n p) d -> p n d", p=128),
                )
                k0 += nblk

            # ---- phase 2: hidden^T = tanh(dense_w^T x + db), k-major ------
            # The db bias matmuls lead the group (start=True on the first
            # clears the bank) so nothing but the final k-block's 8 matmuls
            # remains after the last dense_w DMA lands.
            ph = ps.tile([128, KH, B_LOC], F32, tag="ph")
            for h in range(KH):
                nc.tensor.matmul(
                    ph[:, h, :],
                    db_t[0:1, h * 128 : h * 128 + 128],
                    ones_t[0:1, :],
                    start=(h == 0),
                    stop=False,
                )
            for k in range(KD):
                r, s = divmod(k, 8)
                mov = xt[:, s, r::2]
                for h in range(KH):
                    nc.tensor.matmul(
                        ph[:, h, :],
                        dw_t[:, k, h * 128 : h * 128 + 128],
                        mov,
                        start=False,
                        stop=(k == KD - 1),
                    )
            # ph holds ALPHA*(x @ dense_w + db); the activation's exact
            # power-of-two scale undoes the fp8 weight pre-scale
            ht = consts.tile([128, KH, B_LOC], F16, tag="ht")
            nc.scalar.activation(
                ht[:], ph[:], mybir.ActivationFunctionType.Tanh, scale=1.0 / ALPHA
            )

            # ---- phase 3: logits^T = out_w^T h + ob -----------------------
            pl = ps.tile([2, B_LOC], F32, tag="pl")
            for h in range(KH):
                nc.tensor.matmul(
                    pl[:], ow_t[:, h, :], ht[:, h, :], start=(h == 0), stop=False
                )
            nc.tensor.matmul(
                pl[:], ob_t[0:1, :], ones_t[0:1, :], start=False, stop=True
            )
            lg = consts.tile([2, B_LOC], F32, tag="lg")
            nc.vector.tensor_copy(lg[:], pl[:])
            nc.sync.dma_start(out=out.ap(), in_=lg[:])

    _split_excess_waits(nc)
    if trim:
        try:
            _trim_framework_sync(nc, do_prologue=True, do_epilogue=True)
        except Exception:
            # the trim is a pure optimization; an unexpected program shape
            # must not break the build (rebuild untrimmed)
            return _build_program(T, rows_last, trim=False)
    return nc


def _trim_framework_sync(nc, do_prologue=True, do_epilogue=True):
    """Post-pass on the Tile-generated program:
    1. Remove the prologue all-engine barrier — every body dependency is
       already semaphore-protected, and each engine's own setup precedes
       its body in program order.
    2. Drop the epilogue drain's waits on DMA-lane semaphores that were
       already consumed by in-body readers; only the final (output) DMA's
       lane has no in-body consumer, so only its wait is load-bearing.
    The epilogue barrier + semaphore clears are kept so repeat launches
    still start from clean semaphore state."""
    f = nc.m.functions[0]
    pre, body, epi = f.blocks[0], f.blocks[1], f.blocks[-1]

    removed_update_sems = set()
    out = []
    for inst in pre.instructions if do_prologue else []:
        if isinstance(inst, mybir.InstEventSemaphore) and inst.name.startswith(
            "barrier_"
        ):
            if inst.sync_info:
                removed_update_sems.update(u.id for u in inst.sync_info.on_update)
            continue
        if isinstance(inst, mybir.InstDrain) and inst.sync_info:
            # strip the barrier's waits AND its sem increments together;
            # leaving the increments corrupts the epilogue barrier counts
            inst.sync_info = None
        out.append(inst)
    if do_prologue:
        pre.instructions = out

    # epilogue: rebuild it minimally.  The only load-bearing wait is the
    # output DMA's lane semaphore (every other DMA's semaphore was already
    # consumed by an in-body reader, and the out DMA transitively follows
    # all compute), so the epilogue becomes: SP drain waiting that
    # semaphore, the semaphore RANGE_CLEAR on SP right after it (clean
    # state for repeat launches), and one sync-free pipeline drain per
    # other engine.  Both Tile all-engine barrier rounds are dead weight.
    if do_epilogue:
        dmas = [i for i in body.instructions if isinstance(i, mybir.InstDMACopy)]
        out_upd = dmas[-1].sync_info.on_update[0]
        out_sem = out_upd.id
        final_val = sum(
            int(u.update_value or 0)
            for d in dmas
            for u in (d.sync_info.on_update if d.sync_info else [])
            if u.id == out_sem
        )
        out_wait = None
        for inst in epi.instructions:
            for w in inst.sync_info.on_wait if inst.sync_info else []:
                if w.id == out_sem:
                    out_wait = w
        assert out_wait is not None and int(out_wait.wait_value) == final_val, (
            f"epilogue lacks a wait for the out DMA sem {out_sem} at {final_val}"
        )
        sp_drain = None
        clear_inst = None
        eng_drains = {}
        for inst in epi.instructions:
            tn = type(inst).__name__
            if tn == "InstISA" and clear_inst is None:
                clear_inst = inst
            if tn == "InstDrain":
                key = str(inst.engine)
                if "SP" in key and sp_drain is None:
                    sp_drain = inst
                elif "SP" not in key:
                    eng_drains.setdefault(key, inst)
        assert sp_drain is not None and clear_inst is not None
        sp_drain.sync_info = bass_rust.SyncInfo(on_wait=[out_wait], on_update=[])
        clear_inst.engine = sp_drain.engine
        rebuilt = [sp_drain, clear_inst]
        for inst in eng_drains.values():
            inst.sync_info = None
            rebuilt.append(inst)
        epi.instructions = rebuilt

    # sanity: every waited (sem, value) must be coverable by total updates,
    # and nothing may wait on a semaphore whose barrier update was removed
    from collections import defaultdict

    updates = defaultdict(int)
    waited = defaultdict(int)
    for blk in f.blocks:
        for inst in blk.instructions:
            si = inst.sync_info
            if not si:
                continue
            for u in si.on_update:
                updates[u.id] += int(u.update_value or 0)
            for w in si.on_wait:
                if w.wait_value is not None and "ge" in str(w.wait_mode):
                    waited[w.id] = max(waited[w.id], int(w.wait_value))
    for sem_id, val in waited.items():
        assert updates[sem_id] >= val, (
            f"sem {sem_id}: waits up to {val} but only {updates[sem_id]} updates"
        )
        assert sem_id not in removed_update_sems or updates[sem_id] >= val


_PROGRAM_CACHE = {}
LAST_RESULTS = None


def kernel(embs, input_ids, dense_w, dense_b, out_w, out_b):
    embs = np.ascontiguousarray(np.asarray(embs, dtype=np.float32))
    ids = np.asarray(input_ids)
    dense_w = np.asarray(dense_w, dtype=np.float32)
    dense_b = np.asarray(dense_b, dtype=np.float32)
    out_w = np.asarray(out_w, dtype=np.float32)
    out_b = np.asarray(out_b, dtype=np.float32)

    # host-side mask metadata — exactly the reference's argmax semantics
    idx1 = np.argmax(ids == EOS_ID, axis=-1)
    idx2 = np.argmax(ids == PAD_ID, axis=-1) - 1
    pos = np.arange(S)
    m1 = ((pos >= 1) & (pos < idx1[:, None])).astype(np.float32)
    m2 = ((pos >= idx1[:, None] + 2) & (pos < idx2[:, None])).astype(np.float32)
    n1 = m1.sum(-1, keepdims=True)
    n2 = m2.sum(-1, keepdims=True)
    # empty segments give 0/0 = NaN in the reference; keep device weights
    # finite (zero) and reinstate the NaN on the host afterwards
    w1 = np.where(n1 > 0, m1 / np.maximum(n1, 1), 0.0).astype(np.float32)
    w2 = np.where(n2 > 0, m2 / np.maximum(n2, 1), 0.0).astype(np.float32)
    nan_rows = (n1[:, 0] == 0) | (n2[:, 0] == 0)

    used = (m1 + m2) > 0                      # [B, S] tokens with weight
    tokens = used.sum(axis=1).astype(int)

    # balance samples across cores by exact token count (8 per core)
    order = np.argsort(-tokens, kind="stable")
    loads = np.zeros(N_CORES, dtype=int)
    counts = np.zeros(N_CORES, dtype=int)
    assign = [[] for _ in range(N_CORES)]
    for b in order:
        free = np.nonzero(counts < B_LOC)[0]
        core = free[np.argmin(loads[free])]
        assign[core].append(int(b))
        loads[core] += tokens[b]
        counts[core] += 1
    # pairwise-swap refinement: the exactly-8-per-core constraint leaves the
    # greedy pass a few hundred tokens imbalanced, which costs whole
    # 128-token chunks of stream time on every core
    for _ in range(200):
        i = int(np.argmax(loads))
        best = None
        for j in range(N_CORES):
            if j == i:
                continue
            for ai, a in enumerate(assign[i]):
                for bj, b in enumerate(assign[j]):
                    d = tokens[a] - tokens[b]
                    if d <= 0:
                        continue
                    new_max = max(loads[i] - d, loads[j] + d)
                    if new_max < loads[i] and (best is None or new_max < best[0]):
                        best = (new_max, j, ai, bj)
        if best is None:
            break
        _, j, ai, bj = best
        a, b = assign[i][ai], assign[j][bj]
        assign[i][ai], assign[j][bj] = b, a
        loads[i] += tokens[b] - tokens[a]
        loads[j] += tokens[a] - tokens[b]
    T = max(1, int(-(-loads.max() // 128)))
    rows_last = max(1, int(loads.max() - (T - 1) * 128))

    key = (T, rows_last)
    if key not in _PROGRAM_CACHE:
        _PROGRAM_CACHE[key] = _build_program(T, rows_last)
    nc = _PROGRAM_CACHE[key]

    dw_b = (dense_w * ALPHA).astype(NP_E3M4)
    ow_b = np.ascontiguousarray(
        out_w.reshape(KH, 128, 2).transpose(1, 0, 2).reshape(128, KH * 2)
    ).astype(NP_BF16)
    db_r = (dense_b * ALPHA).reshape(1, D)
    ob_r = out_b.reshape(1, 2)
    ones = np.ones((1, B_LOC), np.float32)

    in_maps = []
    for core in range(N_CORES):
        packed = np.zeros((T * 128, D), dtype=NP_E3M4)
        wmf = np.zeros((T * 128, 16), dtype=np.float32)
        off = 0
        for j, b in enumerate(assign[core]):
            posb = np.nonzero(used[b])[0]
            L = len(posb)
            if L:
                packed[off : off + L] = embs[b, posb].astype(NP_E3M4)
                wmf[off : off + L, 2 * j] = w1[b, posb]
                wmf[off : off + L, 2 * j + 1] = w2[b, posb]
            off += L
        wm16 = np.ascontiguousarray(
            wmf.reshape(T, 128, 16).transpose(1, 0, 2).reshape(128, T * 16)
        ).astype(np.float16)
        in_maps.append(
            {
                "embs": packed,
                "wm": wm16,
                "dw": dw_b,
                "db": db_r,
                "ow": ow_b,
                "ob": ob_r,
                "ones": ones,
            }
        )

    res = run_bass_kernel_spmd(nc, in_maps, list(range(N_CORES)))
    global LAST_RESULTS
    LAST_RESULTS = res

    logits = np.empty((B, 2), np.float32)
    for core in range(N_CORES):
        logits[assign[core]] = res.results[core]["out"].T
    logits[nan_rows] = np.nan
    return logits


# revision 22
# speedup vs baseline: 1.0233x; 1.0059x over previous
"""Trainium2 Bass kernel for BilingualSentenceClassifier (segment_reduce).

Computes, for B=64 samples of S=2048 tokens with D=1024 embedding dims:
  sent1 = mean(embs[1:idx1])            (idx1 = first EOS position)
  sent2 = mean(embs[idx1+2:idx2])       (idx2 = first PAD position - 1)
  logits = tanh(concat(sent1, sent2) @ dense_w + dense_b) @ out_w + out_b

Strategy: pure data parallel over 8 NeuronCores (8 samples per core).
The kernel is HBM-bandwidth bound, so the embedding stream is cut to the
minimum: only tokens that carry nonzero mask weight are shipped, packed
back-to-back at token granularity (samples balanced across cores by exact
token count), and quantized to fp8 e3m4 (the segment means + dense head
keep ~9e-3 relative error, well under the 2e-2 gate).  dense_w streams in
fp8 e3m4 too, pre-scaled by an exact power of two that the tanh
activation's scale input undoes.  Mask weights (1/n at member tokens)
ride in fp16 as the matmul moving operand, so the per-sample
normalization is exact to fp16.

Phase 1 uses the embedding chunk as the *stationary* operand ([128 tok,
128 dims] slices) against the [128 tok, 16] weight matrix, producing the
segment means directly transposed ([dim, 2*sample]) in a single PSUM
bank, which feeds the dense head with no transpose stage: the head runs
with dense_w blocks stationary and [128, 8] moving slices, dense_w
streaming *after* the embeddings so the head chases the tail of the DMA
stream.  Everything downstream of the segment sums stays in fp16/fp32.
"""

import sys

sys.path.insert(0, "/opt/trn_rl_repo")

import numpy as np
import ml_dtypes

import concourse.bass as bass
import concourse.tile as tile
from concourse import mybir
import bass_rust
from concourse.bass_utils import run_bass_kernel_spmd

B, S, D = 64, 2048, 1024
EOS_ID, PAD_ID = 2, 1
N_CORES = 8
B_LOC = B // N_CORES          # samples per core
KD = 16                       # 128-row contraction blocks in dense_w
KH = D // 128                 # 128-row contraction blocks in out_w
G = 16                        # sequence chunks per embedding DMA
ALPHA = 128.0                 # dense_w fp8 pre-scale (exact power of two)

F32 = mybir.dt.float32
F32R = mybir.dt.float32r
BF16 = mybir.dt.bfloat16
F16 = mybir.dt.float16
F8E3 = mybir.dt.float8e3

NP_E3M4 = ml_dtypes.float8_e3m4
NP_BF16 = ml_dtypes.bfloat16


def _split_excess_waits(nc, max_waits=1):
    """This container's walrus rejects instructions carrying more than 1-2
    sync waits (e.g. the Tile tail drain).  Hoist excess waits onto
    preceding same-engine NOPs — semantically identical: the engine's
    sequencer blocks on the NOP's wait before dispatching the original
    instruction."""
    cnt = 0
    for f in nc.m.functions:
        for blk in f.blocks:
            out = []
            changed = False
            for inst in blk.instructions:
                si = inst.sync_info
                if si is not None and len(si.on_wait) > max_waits:
                    waits = list(si.on_wait)
                    for w in waits[:-max_waits]:
                        cnt += 1
                        nop = mybir.InstNoOp(name=f"{inst.name}-hw{cnt}")
                        nop.engine = inst.engine
                        nop.sync_info = bass_rust.SyncInfo(on_wait=[w], on_update=[])
                        out.append(nop)
                    inst.sync_info = bass_rust.SyncInfo(
                        on_wait=waits[-max_waits:], on_update=list(si.on_update)
                    )
                    changed = True
                out.append(inst)
            if changed:
                blk.instructions = out
    return cnt


def _build_program_untrimmed(T, rows_last):
    nc = _build_program(T, rows_last, trim=False)
    return nc


def _build_program(T, rows_last, trim=True):
    """SPMD program processing T 128-token chunks of packed embeddings; the
    final chunk only carries rows_last valid token rows."""
    nc = bass.Bass("TRN2", target_bir_lowering=False, debug=False, num_devices=N_CORES)

    embs = nc.dram_tensor("embs", [T * 128, D], F8E3, kind="ExternalInput")
    wm = nc.dram_tensor("wm", [128, T * 16], F16, kind="ExternalInput")
    dw = nc.dram_tensor("dw", [2 * D, D], F8E3, kind="ExternalInput")
    db = nc.dram_tensor("db", [1, D], F32R, kind="ExternalInput")
    # ow pre-packed on host to [128, KH*2] (partition-major) so the DMA
    # moves one 32B run per partition instead of 2048 4-byte scatters
    ow = nc.dram_tensor("ow", [128, KH * 2], BF16, kind="ExternalInput")
    ob = nc.dram_tensor("ob", [1, 2], F32R, kind="ExternalInput")
    ones = nc.dram_tensor("ones", [1, B_LOC], F32R, kind="ExternalInput")
    out = nc.dram_tensor("out", [2, B_LOC], F32, kind="ExternalOutput")

    groups = []
    t0 = 0
    while t0 < T:
        groups.append((t0, min(G, T - t0)))
        t0 += min(G, T - t0)

    with tile.TileContext(nc) as tc:
        with (
            tc.tile_pool(name="sb", bufs=1) as consts,
            tc.tile_pool(name="ps", bufs=1, space="PSUM") as ps,
        ):
            embp = dwp = consts
            # ---- phase 1: segment sums, directly transposed ---------------
            # xt_ps[p, s, q] = sum_tok emb[tok, 128 s + p] * wm[tok, q]
            # (q = 2 j + r selects sample j / segment r; wm carries 1/n).
            # All 8 dim-slices accumulate into one PSUM bank: start=True only
            # on the very first matmul (clears the bank's has_written bits);
            # every later first-touch overwrites-where-unset, then
            # accumulates.
            # The first embedding group's DMA is issued before the params so
            # the param DMAs' issue overhead hides under its transfer.
            xt_ps = ps.tile([128, 8, 16], F32, tag="xt_ps")
            wm_t = consts.tile([128, T, 16], F16, tag="wm")
            ow_t = consts.tile([128, KH, 2], BF16, tag="ow")
            db_t = consts.tile([1, D], F32R, tag="db")
            ob_t = consts.tile([1, 2], F32R, tag="ob")
            ones_t = consts.tile([1, B_LOC], F32R, tag="ones")
            warm = consts.tile([1, 8], F32, tag="warm")
            for g, (gt, gn) in enumerate(groups):
                et = embp.tile([128, gn, D], F8E3, tag=f"emb{g}")
                nfull = gn if (gt + gn < T or rows_last == 128) else gn - 1
                if nfull:
                    src = embs.ap()[gt * 128 : (gt + nfull) * 128, :]
                    nc.sync.dma_start(
                        out=et[:, :nfull, :],
                        in_=src.rearrange("(n p) d -> p n d", p=128),
                    )
                if nfull < gn:
                    base = (gt + nfull) * 128
                    nc.sync.dma_start(
                        out=et[:rows_last, nfull, :],
                        in_=embs.ap()[base : base + rows_last, :],
                    )
                if g == 0:
                    nc.sync.dma_start(out=wm_t[:], in_=wm.ap())
                    nc.sync.dma_start(out=ow_t[:], in_=ow.ap())
                    nc.sync.dma_start(out=db_t[:], in_=db.ap())
                    nc.sync.dma_start(out=ob_t[:], in_=ob.ap())
                    nc.sync.dma_start(out=ones_t[:], in_=ones.ap())
                    # warm the ScalarE Tanh LUT while the stream runs
                    nc.vector.memset(warm[:], 0.0)
                    nc.scalar.activation(
                        warm[:], warm[:], mybir.ActivationFunctionType.Tanh
                    )
                for c in range(gn):
                    t = gt + c
                    rows = 128 if t < T - 1 else rows_last
                    for s in range(8):
                        nc.tensor.matmul(
                            xt_ps[:, s, :],
                            et[0:rows, c, s * 128 : s * 128 + 128],
                            wm_t[0:rows, t, :],
                            start=(t == 0 and s == 0),
                            stop=(t == T - 1),
                        )
            xt = consts.tile([128, 8, 16], F16, tag="xt")
            nc.vector.tensor_copy(xt[:], xt_ps[:])

            # dense_w streams after the embeddings; the head chases it.
            # Batched into >=3-block DMAs (transfer > the 625ns issue cost)
            # with a small final DMA so the post-stream matmul tail is short.
            dw_t = dwp.tile([128, KD, D], F8E3, tag="dw")
            k0 = 0
            for nblk in (3, 3, 3, 3, 3, 1):
                src = dw.ap()[128 * k0 : 128 * (k0 + nblk), :]
                nc.sync.dma_start(
                    out=dw_t[:, k0 : k0 + nblk, :],
                    in_=src.rearrange("(n p) d -> p n d", p=128),
                )
                k0 += nblk

            # ---- phase 2: hidden^T = tanh(dense_w^T x + db), k-major ------
            # The db bias matmuls lead the group (start=True on the first
            # clears the bank) so nothing but the final k-block's 8 matmuls
            # remains after the last dense_w DMA lands.
            ph = ps.tile([128, KH, B_LOC], F32, tag="ph")
            for h in range(KH):
                nc.tensor.matmul(
                    ph[:, h, :],
                    db_t[0:1, h * 128 : h * 128 + 128],
                    ones_t[0:1, :],
                    start=(h == 0),
                    stop=False,
                )
            for k in range(KD):
                r, s = divmod(k, 8)
                mov = xt[:, s, r::2]
                for h in range(KH):
                    nc.tensor.matmul(
                        ph[:, h, :],
                        dw_t[:, k, h * 128 : h * 128 + 128],
                        mov,
                        start=False,
                        stop=(k == KD - 1),
                    )
            # ph holds ALPHA*(x @ dense_w + db); the activation's exact
            # power-of-two scale undoes the fp8 weight pre-scale
            ht = consts.tile([128, KH, B_LOC], F16, tag="ht")
            nc.scalar.activation(
                ht[:], ph[:], mybir.ActivationFunctionType.Tanh, scale=1.0 / ALPHA
            )

            # ---- phase 3: logits^T = out_w^T h + ob -----------------------
            pl = ps.tile([2, B_LOC], F32, tag="pl")
            for h in range(KH):
                nc.tensor.matmul(
                    pl[:], ow_t[:, h, :], ht[:, h, :], start=(h == 0), stop=False
                )
            nc.tensor.matmul(
                pl[:], ob_t[0:1, :], ones_t[0:1, :], start=False, stop=True
            )
            lg = consts.tile([2, B_LOC], F32, tag="lg")
            nc.vector.tensor_copy(lg[:], pl[:])
            nc.sync.dma_start(out=out.ap(), in_=lg[:])

    _split_excess_waits(nc)
    if trim:
        try:
            _trim_framework_sync(nc, do_prologue=True, do_epilogue=True)
        except Exception:
            # the trim is a pure optimization; an unexpected program shape
            # must not break the build (rebuild untrimmed)
            return _build_program(T, rows_last, trim=False)
    return nc


def _trim_framework_sync(nc, do_prologue=True, do_epilogue=True):
    """Post-pass on the Tile-generated program:
    1. Remove the prologue all-engine barrier — every body dependency is
       already semaphore-protected, and each engine's own setup precedes
       its body in program order.
    2. Drop the epilogue drain's waits on DMA-lane semaphores that were
       already consumed by in-body readers; only the final (output) DMA's
       lane has no in-body consumer, so only its wait is load-bearing.
    The epilogue barrier + semaphore clears are kept so repeat launches
    still start from clean semaphore state."""
    f = nc.m.functions[0]
    pre, body, epi = f.blocks[0], f.blocks[1], f.blocks[-1]

    removed_update_sems = set()
    out = []
    for inst in pre.instructions if do_prologue else []:
        if isinstance(inst, mybir.InstRegisterMove):
            # zero/broadcast register init; nothing in this program reads
            # registers (all access patterns are static)
            continue
        if isinstance(inst, mybir.InstEventSemaphore) and inst.name.startswith(
            "barrier_"
        ):
            if inst.sync_info:
                removed_update_sems.update(u.id for u in inst.sync_info.on_update)
            continue
        if isinstance(inst, mybir.InstDrain) and inst.sync_info:
            # strip the barrier's waits AND its sem increments together;
            # leaving the increments corrupts the epilogue barrier counts
            inst.sync_info = None
        out.append(inst)
    if do_prologue:
        pre.instructions = out

    # epilogue: rebuild it minimally.  The only load-bearing wait is the
    # output DMA's lane semaphore (every other DMA's semaphore was already
    # consumed by an in-body reader, and the out DMA transitively follows
    # all compute), so the epilogue becomes: SP drain waiting that
    # semaphore, the semaphore RANGE_CLEAR on SP right after it (clean
    # state for repeat launches), and one sync-free pipeline drain per
    # other engine.  Both Tile all-engine barrier rounds are dead weight.
    if do_epilogue:
        dmas = [i for i in body.instructions if isinstance(i, mybir.InstDMACopy)]
        out_upd = dmas[-1].sync_info.on_update[0]
        out_sem = out_upd.id
        final_val = sum(
            int(u.update_value or 0)
            for d in dmas
            for u in (d.sync_info.on_update if d.sync_info else [])
            if u.id == out_sem
        )
        out_wait = None
        for inst in epi.instructions:
            for w in inst.sync_info.on_wait if inst.sync_info else []:
                if w.id == out_sem:
                    out_wait = w
        assert out_wait is not None and int(out_wait.wait_value) == final_val, (
            f"epilogue lacks a wait for the out DMA sem {out_sem} at {final_val}"
        )
        sp_drain = None
        clear_inst = None
        eng_drains = {}
        for inst in epi.instructions:
            tn = type(inst).__name__
            if tn == "InstISA" and clear_inst is None:
                clear_inst = inst
            if tn == "InstDrain":
                key = str(inst.engine)
                if "SP" in key and sp_drain is None:
                    sp_drain = inst
                elif "SP" not in key:
                    eng_drains.setdefault(key, inst)
        assert sp_drain is not None and clear_inst is not None
        sp_drain.sync_info = bass_rust.SyncInfo(on_wait=[out_wait], on_update=[])
        clear_inst.engine = sp_drain.engine
        rebuilt = [sp_drain, clear_inst]
        for inst in eng_drains.values():
            inst.sync_info = None
            rebuilt.append(inst)
        epi.instructions = rebuilt

    # sanity: every waited (sem, value) must be coverable by total updates,
    # and nothing may wait on a semaphore whose barrier update was removed
    from collections import defaultdict

    updates = defaultdict(int)
    waited = defaultdict(int)
    for blk in f.blocks:
        for inst in blk.instructions:
            si = inst.sync_info
            if not si:
                continue
            for u in si.on_update:
                updates[u.id] += int(u.update_value or 0)
            for w in si.on_wait:
                if w.wait_value is not None and "ge" in str(w.wait_mode):
                    waited[w.id] = max(waited[w.id], int(w.wait_value))
    for sem_id, val in waited.items():
        assert updates[sem_id] >= val, (
            f"sem {sem_id}: waits up to {val} but only {updates[sem_id]} updates"
        )
        assert sem_id not in removed_update_sems or updates[sem_id] >= val


_PROGRAM_CACHE = {}
LAST_RESULTS = None


def kernel(embs, input_ids, dense_w, dense_b, out_w, out_b):
    embs = np.ascontiguousarray(np.asarray(embs, dtype=np.float32))
    ids = np.asarray(input_ids)
    dense_w = np.asarray(dense_w, dtype=np.float32)
    dense_b = np.asarray(dense_b, dtype=np.float32)
    out_w = np.asarray(out_w, dtype=np.float32)
    out_b = np.asarray(out_b, dtype=np.float32)

    # host-side mask metadata — exactly the reference's argmax semantics
    idx1 = np.argmax(ids == EOS_ID, axis=-1)
    idx2 = np.argmax(ids == PAD_ID, axis=-1) - 1
    pos = np.arange(S)
    m1 = ((pos >= 1) & (pos < idx1[:, None])).astype(np.float32)
    m2 = ((pos >= idx1[:, None] + 2) & (pos < idx2[:, None])).astype(np.float32)
    n1 = m1.sum(-1, keepdims=True)
    n2 = m2.sum(-1, keepdims=True)
    # empty segments give 0/0 = NaN in the reference; keep device weights
    # finite (zero) and reinstate the NaN on the host afterwards
    w1 = np.where(n1 > 0, m1 / np.maximum(n1, 1), 0.0).astype(np.float32)
    w2 = np.where(n2 > 0, m2 / np.maximum(n2, 1), 0.0).astype(np.float32)
    nan_rows = (n1[:, 0] == 0) | (n2[:, 0] == 0)

    used = (m1 + m2) > 0                      # [B, S] tokens with weight
    tokens = used.sum(axis=1).astype(int)

    # balance samples across cores by exact token count (8 per core)
    order = np.argsort(-tokens, kind="stable")
    loads = np.zeros(N_CORES, dtype=int)
    counts = np.zeros(N_CORES, dtype=int)
    assign = [[] for _ in range(N_CORES)]
    for b in order:
        free = np.nonzero(counts < B_LOC)[0]
        core = free[np.argmin(loads[free])]
        assign[core].append(int(b))
        loads[core] += tokens[b]
        counts[core] += 1
    # pairwise-swap refinement: the exactly-8-per-core constraint leaves the
    # greedy pass a few hundred tokens imbalanced, which costs whole
    # 128-token chunks of stream time on every core
    for _ in range(200):
        i = int(np.argmax(loads))
        best = None
        for j in range(N_CORES):
            if j == i:
                continue
            for ai, a in enumerate(assign[i]):
                for bj, b in enumerate(assign[j]):
                    d = tokens[a] - tokens[b]
                    if d <= 0:
                        continue
                    new_max = max(loads[i] - d, loads[j] + d)
                    if new_max < loads[i] and (best is None or new_max < best[0]):
                        best = (new_max, j, ai, bj)
        if best is None:
            break
        _, j, ai, bj = best
        a, b = assign[i][ai], assign[j][bj]
        assign[i][ai], assign[j][bj] = b, a
        loads[i] += tokens[b] - tokens[a]
        loads[j] += tokens[a] - tokens[b]
    T = max(1, int(-(-loads.max() // 128)))
    rows_last = max(1, int(loads.max() - (T - 1) * 128))

    key = (T, rows_last)
    if key not in _PROGRAM_CACHE:
        _PROGRAM_CACHE[key] = _build_program(T, rows_last)
    nc = _PROGRAM_CACHE[key]

    dw_b = (dense_w * ALPHA).astype(NP_E3M4)
    ow_b = np.ascontiguousarray(
        out_w.reshape(KH, 128, 2).transpose(1, 0, 2).reshape(128, KH * 2)
    ).astype(NP_BF16)
    db_r = (dense_b * ALPHA).reshape(1, D)
    ob_r = out_b.reshape(1, 2)
    ones = np.ones((1, B_LOC), np.float32)

    in_maps = []
    for core in range(N_CORES):
        packed = np.zeros((T * 128, D), dtype=NP_E3M4)
        wmf = np.zeros((T * 128, 16), dtype=np.float32)
        off = 0
        for j, b in enumerate(assign[core]):
            posb = np.nonzero(used[b])[0]
            L = len(posb)
            if L:
                packed[off : off + L] = embs[b, posb].astype(NP_E3M4)
                wmf[off : off + L, 2 * j] = w1[b, posb]
                wmf[off : off + L, 2 * j + 1] = w2[b, posb]
            off += L
        wm16 = np.ascontiguousarray(
            wmf.reshape(T, 128, 16).transpose(1, 0, 2).reshape(128, T * 16)
        ).astype(np.float16)
        in_maps.append(
            {
                "embs": packed,
                "wm": wm16,
                "dw": dw_b,
                "db": db_r,
                "ow": ow_b,
                "ob": ob_r,
                "ones": ones,
            }
        )

    res = run_bass_kernel_spmd(nc, in_maps, list(range(N_CORES)))
    global LAST_RESULTS
    LAST_RESULTS = res

    logits = np.empty((B, 2), np.float32)
    for core in range(N_CORES):
        logits[assign[core]] = res.results[core]["out"].T
    logits[nan_rows] = np.nan
    return logits


# revision 23
# speedup vs baseline: 1.0239x; 1.0006x over previous
"""Trainium2 Bass kernel for BilingualSentenceClassifier (segment_reduce).

Computes, for B=64 samples of S=2048 tokens with D=1024 embedding dims:
  sent1 = mean(embs[1:idx1])            (idx1 = first EOS position)
  sent2 = mean(embs[idx1+2:idx2])       (idx2 = first PAD position - 1)
  logits = tanh(concat(sent1, sent2) @ dense_w + dense_b) @ out_w + out_b

Strategy: pure data parallel over 8 NeuronCores (8 samples per core).
The kernel is HBM-bandwidth bound, so the embedding stream is cut to the
minimum: only tokens that carry nonzero mask weight are shipped, packed
back-to-back at token granularity (samples balanced across cores by exact
token count), and quantized to fp8 e3m4 (the segment means + dense head
keep ~9e-3 relative error, well under the 2e-2 gate).  dense_w streams in
fp8 e3m4 too, pre-scaled by an exact power of two that the tanh
activation's scale input undoes.  Mask weights (1/n at member tokens)
ride in fp16 as the matmul moving operand, so the per-sample
normalization is exact to fp16.

Phase 1 uses the embedding chunk as the *stationary* operand ([128 tok,
128 dims] slices) against the [128 tok, 16] weight matrix, producing the
segment means directly transposed ([dim, 2*sample]) in a single PSUM
bank, which feeds the dense head with no transpose stage: the head runs
with dense_w blocks stationary and [128, 8] moving slices, dense_w
streaming *after* the embeddings so the head chases the tail of the DMA
stream.  Everything downstream of the segment sums stays in fp16/fp32.
"""

import sys

sys.path.insert(0, "/opt/trn_rl_repo")

import numpy as np
import ml_dtypes

import concourse.bass as bass
import concourse.tile as tile
from concourse import mybir
import bass_rust
from concourse.bass_utils import run_bass_kernel_spmd

B, S, D = 64, 2048, 1024
EOS_ID, PAD_ID = 2, 1
N_CORES = 8
B_LOC = B // N_CORES          # samples per core
KD = 16                       # 128-row contraction blocks in dense_w
KH = D // 128                 # 128-row contraction blocks in out_w
G = 16                        # sequence chunks per embedding DMA
ALPHA = 128.0                 # dense_w fp8 pre-scale (exact power of two)

F32 = mybir.dt.float32
F32R = mybir.dt.float32r
BF16 = mybir.dt.bfloat16
F16 = mybir.dt.float16
F8E3 = mybir.dt.float8e3

NP_E3M4 = ml_dtypes.float8_e3m4
NP_BF16 = ml_dtypes.bfloat16


def _split_excess_waits(nc, max_waits=1):
    """This container's walrus rejects instructions carrying more than 1-2
    sync waits (e.g. the Tile tail drain).  Hoist excess waits onto
    preceding same-engine NOPs — semantically identical: the engine's
    sequencer blocks on the NOP's wait before dispatching the original
    instruction."""
    cnt = 0
    for f in nc.m.functions:
        for blk in f.blocks:
            out = []
            changed = False
            for inst in blk.instructions:
                si = inst.sync_info
                if si is not None and len(si.on_wait) > max_waits:
                    waits = list(si.on_wait)
                    for w in waits[:-max_waits]:
                        cnt += 1
                        nop = mybir.InstNoOp(name=f"{inst.name}-hw{cnt}")
                        nop.engine = inst.engine
                        nop.sync_info = bass_rust.SyncInfo(on_wait=[w], on_update=[])
                        out.append(nop)
                    inst.sync_info = bass_rust.SyncInfo(
                        on_wait=waits[-max_waits:], on_update=list(si.on_update)
                    )
                    changed = True
                out.append(inst)
            if changed:
                blk.instructions = out
    return cnt


def _build_program_untrimmed(T, rows_last):
    nc = _build_program(T, rows_last, trim=False)
    return nc


def _build_program(T, rows_last, trim=True):
    """SPMD program processing T 128-token chunks of packed embeddings; the
    final chunk only carries rows_last valid token rows."""
    nc = bass.Bass("TRN2", target_bir_lowering=False, debug=False, num_devices=N_CORES)

    embs = nc.dram_tensor("embs", [T * 128, D], F8E3, kind="ExternalInput")
    wm = nc.dram_tensor("wm", [128, T * 16], F16, kind="ExternalInput")
    dw = nc.dram_tensor("dw", [2 * D, D], F8E3, kind="ExternalInput")
    db = nc.dram_tensor("db", [1, D], F32R, kind="ExternalInput")
    # ow pre-packed on host to [128, KH*2] (partition-major) so the DMA
    # moves one 32B run per partition instead of 2048 4-byte scatters
    ow = nc.dram_tensor("ow", [128, KH * 2], BF16, kind="ExternalInput")
    ob = nc.dram_tensor("ob", [1, 2], F32R, kind="ExternalInput")
    ones = nc.dram_tensor("ones", [1, B_LOC], F32R, kind="ExternalInput")
    out = nc.dram_tensor("out", [2, B_LOC], F32, kind="ExternalOutput")

    groups = []
    t0 = 0
    while t0 < T:
        groups.append((t0, min(G, T - t0)))
        t0 += min(G, T - t0)

    with tile.TileContext(nc) as tc:
        with (
            tc.tile_pool(name="sb", bufs=1) as consts,
            tc.tile_pool(name="ps", bufs=1, space="PSUM") as ps,
        ):
            embp = dwp = consts
            # ---- phase 1: segment sums, directly transposed ---------------
            # xt_ps[p, s, q] = sum_tok emb[tok, 128 s + p] * wm[tok, q]
            # (q = 2 j + r selects sample j / segment r; wm carries 1/n).
            # All 8 dim-slices accumulate into one PSUM bank: start=True only
            # on the very first matmul (clears the bank's has_written bits);
            # every later first-touch overwrites-where-unset, then
            # accumulates.
            # The first embedding group's DMA is issued before the params so
            # the param DMAs' issue overhead hides under its transfer.
            xt_ps = ps.tile([128, 8, 16], F32, tag="xt_ps")
            wm_t = consts.tile([128, T, 16], F16, tag="wm")
            ow_t = consts.tile([128, KH, 2], BF16, tag="ow")
            db_t = consts.tile([1, D], F32R, tag="db")
            ob_t = consts.tile([1, 2], F32R, tag="ob")
            ones_t = consts.tile([1, B_LOC], F32R, tag="ones")
            warm = consts.tile([1, 8], F32, tag="warm")
            for g, (gt, gn) in enumerate(groups):
                et = embp.tile([128, gn, D], F8E3, tag=f"emb{g}")
                nfull = gn if (gt + gn < T or rows_last == 128) else gn - 1
                if nfull:
                    src = embs.ap()[gt * 128 : (gt + nfull) * 128, :]
                    nc.sync.dma_start(
                        out=et[:, :nfull, :],
                        in_=src.rearrange("(n p) d -> p n d", p=128),
                    )
                if nfull < gn:
                    base = (gt + nfull) * 128
                    nc.sync.dma_start(
                        out=et[:rows_last, nfull, :],
                        in_=embs.ap()[base : base + rows_last, :],
                    )
                if g == 0:
                    nc.sync.dma_start(out=wm_t[:], in_=wm.ap())
                    nc.sync.dma_start(out=ow_t[:], in_=ow.ap())
                    nc.sync.dma_start(out=db_t[:], in_=db.ap())
                    nc.sync.dma_start(out=ob_t[:], in_=ob.ap())
                    nc.sync.dma_start(out=ones_t[:], in_=ones.ap())
                    # warm the ScalarE Tanh LUT while the stream runs
                    nc.vector.memset(warm[:], 0.0)
                    nc.scalar.activation(
                        warm[:], warm[:], mybir.ActivationFunctionType.Tanh
                    )
                for c in range(gn):
                    t = gt + c
                    rows = 128 if t < T - 1 else rows_last
                    for s in range(8):
                        nc.tensor.matmul(
                            xt_ps[:, s, :],
                            et[0:rows, c, s * 128 : s * 128 + 128],
                            wm_t[0:rows, t, :],
                            start=(t == 0 and s == 0),
                            stop=(t == T - 1),
                        )
            xt = consts.tile([128, 8, 16], F16, tag="xt")
            nc.vector.tensor_copy(xt[:], xt_ps[:])

            # dense_w streams after the embeddings; the head chases it.
            # Batched into >=3-block DMAs (transfer > the 625ns issue cost)
            # with a small final DMA so the post-stream matmul tail is short.
            dw_t = dwp.tile([128, KD, D], F8E3, tag="dw")
            k0 = 0
            for nblk in (3, 3, 3, 3, 3, 1):
                src = dw.ap()[128 * k0 : 128 * (k0 + nblk), :]
                nc.sync.dma_start(
                    out=dw_t[:, k0 : k0 + nblk, :],
                    in_=src.rearrange("(n p) d -> p n d", p=128),
                )
                k0 += nblk

            # ---- phase 2: hidden^T = tanh(dense_w^T x + db), k-major ------
            # The db bias matmuls lead the group (start=True on the first
            # clears the bank) so nothing but the final k-block's 8 matmuls
            # remains after the last dense_w DMA lands.
            ph = ps.tile([128, KH, B_LOC], F32, tag="ph")
            for h in range(KH):
                nc.tensor.matmul(
                    ph[:, h, :],
                    db_t[0:1, h * 128 : h * 128 + 128],
                    ones_t[0:1, :],
                    start=(h == 0),
                    stop=False,
                )
            for k in range(KD):
                r, s = divmod(k, 8)
                mov = xt[:, s, r::2]
                for h in range(KH):
                    nc.tensor.matmul(
                        ph[:, h, :],
                        dw_t[:, k, h * 128 : h * 128 + 128],
                        mov,
                        start=False,
                        stop=(k == KD - 1),
                    )
            # ph holds ALPHA*(x @ dense_w + db); the activation's exact
            # power-of-two scale undoes the fp8 weight pre-scale
            ht = consts.tile([128, KH, B_LOC], F16, tag="ht")
            nc.scalar.activation(
                ht[:], ph[:], mybir.ActivationFunctionType.Tanh, scale=1.0 / ALPHA
            )

            # ---- phase 3: logits^T = out_w^T h + ob -----------------------
            pl = ps.tile([2, B_LOC], F32, tag="pl")
            for h in range(KH):
                nc.tensor.matmul(
                    pl[:], ow_t[:, h, :], ht[:, h, :], start=(h == 0), stop=False
                )
            nc.tensor.matmul(
                pl[:], ob_t[0:1, :], ones_t[0:1, :], start=False, stop=True
            )
            lg = consts.tile([2, B_LOC], F32, tag="lg")
            nc.vector.tensor_copy(lg[:], pl[:])
            nc.sync.dma_start(out=out.ap(), in_=lg[:])

    _split_excess_waits(nc)
    if trim:
        try:
            _trim_framework_sync(nc, do_prologue=True, do_epilogue=True)
        except Exception:
            # the trim is a pure optimization; an unexpected program shape
            # must not break the build (rebuild untrimmed)
            return _build_program(T, rows_last, trim=False)
    return nc


def _trim_framework_sync(nc, do_prologue=True, do_epilogue=True):
    """Post-pass on the Tile-generated program:
    1. Remove the prologue all-engine barrier — every body dependency is
       already semaphore-protected, and each engine's own setup precedes
       its body in program order.
    2. Drop the epilogue drain's waits on DMA-lane semaphores that were
       already consumed by in-body readers; only the final (output) DMA's
       lane has no in-body consumer, so only its wait is load-bearing.
    The epilogue barrier + semaphore clears are kept so repeat launches
    still start from clean semaphore state."""
    f = nc.m.functions[0]
    pre, body, epi = f.blocks[0], f.blocks[1], f.blocks[-1]

    removed_update_sems = set()
    out = []
    for inst in pre.instructions if do_prologue else []:
        if isinstance(inst, mybir.InstRegisterMove):
            # zero/broadcast register init; nothing in this program reads
            # registers (all access patterns are static)
            continue
        if isinstance(inst, mybir.InstEventSemaphore) and inst.name.startswith(
            "barrier_"
        ):
            if inst.sync_info:
                removed_update_sems.update(u.id for u in inst.sync_info.on_update)
            continue
        if isinstance(inst, mybir.InstDrain):
            # the prologue drains only existed to serve the barrier
            continue
        out.append(inst)
    if do_prologue:
        pre.instructions = out

    # epilogue: rebuild it minimally.  The only load-bearing wait is the
    # output DMA's lane semaphore (every other DMA's semaphore was already
    # consumed by an in-body reader, and the out DMA transitively follows
    # all compute), so the epilogue becomes: SP drain waiting that
    # semaphore, the semaphore RANGE_CLEAR on SP right after it (clean
    # state for repeat launches), and one sync-free pipeline drain per
    # other engine.  Both Tile all-engine barrier rounds are dead weight.
    if do_epilogue:
        dmas = [i for i in body.instructions if isinstance(i, mybir.InstDMACopy)]
        out_upd = dmas[-1].sync_info.on_update[0]
        out_sem = out_upd.id
        final_val = sum(
            int(u.update_value or 0)
            for d in dmas
            for u in (d.sync_info.on_update if d.sync_info else [])
            if u.id == out_sem
        )
        out_wait = None
        for inst in epi.instructions:
            for w in inst.sync_info.on_wait if inst.sync_info else []:
                if w.id == out_sem:
                    out_wait = w
        assert out_wait is not None and int(out_wait.wait_value) == final_val, (
            f"epilogue lacks a wait for the out DMA sem {out_sem} at {final_val}"
        )
        sp_drain = None
        clear_inst = None
        eng_drains = {}
        for inst in epi.instructions:
            tn = type(inst).__name__
            if tn == "InstISA" and clear_inst is None:
                clear_inst = inst
            if tn == "InstDrain":
                key = str(inst.engine)
                if "SP" in key and sp_drain is None:
                    sp_drain = inst
                elif "SP" not in key:
                    eng_drains.setdefault(key, inst)
        assert sp_drain is not None and clear_inst is not None
        sp_drain.sync_info = bass_rust.SyncInfo(on_wait=[out_wait], on_update=[])
        clear_inst.engine = sp_drain.engine
        rebuilt = [sp_drain, clear_inst]
        for inst in eng_drains.values():
            inst.sync_info = None
            rebuilt.append(inst)
        epi.instructions = rebuilt

    # sanity: every waited (sem, value) must be coverable by total updates,
    # and nothing may wait on a semaphore whose barrier update was removed
    from collections import defaultdict

    updates = defaultdict(int)
    waited = defaultdict(int)
    for blk in f.blocks:
        for inst in blk.instructions:
            si = inst.sync_info
            if not si:
                continue
            for u in si.on_update:
                updates[u.id] += int(u.update_value or 0)
            for w in si.on_wait:
                if w.wait_value is not None and "ge" in str(w.wait_mode):
                    waited[w.id] = max(waited[w.id], int(w.wait_value))
    for sem_id, val in waited.items():
        assert updates[sem_id] >= val, (
            f"sem {sem_id}: waits up to {val} but only {updates[sem_id]} updates"
        )
        assert sem_id not in removed_update_sems or updates[sem_id] >= val


_PROGRAM_CACHE = {}
LAST_RESULTS = None


def kernel(embs, input_ids, dense_w, dense_b, out_w, out_b):
    embs = np.ascontiguousarray(np.asarray(embs, dtype=np.float32))
    ids = np.asarray(input_ids)
    dense_w = np.asarray(dense_w, dtype=np.float32)
    dense_b = np.asarray(dense_b, dtype=np.float32)
    out_w = np.asarray(out_w, dtype=np.float32)
    out_b = np.asarray(out_b, dtype=np.float32)

    # host-side mask metadata — exactly the reference's argmax semantics
    idx1 = np.argmax(ids == EOS_ID, axis=-1)
    idx2 = np.argmax(ids == PAD_ID, axis=-1) - 1
    pos = np.arange(S)
    m1 = ((pos >= 1) & (pos < idx1[:, None])).astype(np.float32)
    m2 = ((pos >= idx1[:, None] + 2) & (pos < idx2[:, None])).astype(np.float32)
    n1 = m1.sum(-1, keepdims=True)
    n2 = m2.sum(-1, keepdims=True)
    # empty segments give 0/0 = NaN in the reference; keep device weights
    # finite (zero) and reinstate the NaN on the host afterwards
    w1 = np.where(n1 > 0, m1 / np.maximum(n1, 1), 0.0).astype(np.float32)
    w2 = np.where(n2 > 0, m2 / np.maximum(n2, 1), 0.0).astype(np.float32)
    nan_rows = (n1[:, 0] == 0) | (n2[:, 0] == 0)

    used = (m1 + m2) > 0                      # [B, S] tokens with weight
    tokens = used.sum(axis=1).astype(int)

    # balance samples across cores by exact token count (8 per core)
    order = np.argsort(-tokens, kind="stable")
    loads = np.zeros(N_CORES, dtype=int)
    counts = np.zeros(N_CORES, dtype=int)
    assign = [[] for _ in range(N_CORES)]
    for b in order:
        free = np.nonzero(counts < B_LOC)[0]
        core = free[np.argmin(loads[free])]
        assign[core].append(int(b))
        loads[core] += tokens[b]
        counts[core] += 1
    # pairwise-swap refinement: the exactly-8-per-core constraint leaves the
    # greedy pass a few hundred tokens imbalanced, which costs whole
    # 128-token chunks of stream time on every core
    for _ in range(200):
        i = int(np.argmax(loads))
        best = None
        for j in range(N_CORES):
            if j == i:
                continue
            for ai, a in enumerate(assign[i]):
                for bj, b in enumerate(assign[j]):
                    d = tokens[a] - tokens[b]
                    if d <= 0:
                        continue
                    new_max = max(loads[i] - d, loads[j] + d)
                    if new_max < loads[i] and (best is None or new_max < best[0]):
                        best = (new_max, j, ai, bj)
        if best is None:
            break
        _, j, ai, bj = best
        a, b = assign[i][ai], assign[j][bj]
        assign[i][ai], assign[j][bj] = b, a
        loads[i] += tokens[b] - tokens[a]
        loads[j] += tokens[a] - tokens[b]
    T = max(1, int(-(-loads.max() // 128)))
    rows_last = max(1, int(loads.max() - (T - 1) * 128))

    key = (T, rows_last)
    if key not in _PROGRAM_CACHE:
        _PROGRAM_CACHE[key] = _build_program(T, rows_last)
    nc = _PROGRAM_CACHE[key]

    dw_b = (dense_w * ALPHA).astype(NP_E3M4)
    ow_b = np.ascontiguousarray(
        out_w.reshape(KH, 128, 2).transpose(1, 0, 2).reshape(128, KH * 2)
    ).astype(NP_BF16)
    db_r = (dense_b * ALPHA).reshape(1, D)
    ob_r = out_b.reshape(1, 2)
    ones = np.ones((1, B_LOC), np.float32)

    in_maps = []
    for core in range(N_CORES):
        packed = np.zeros((T * 128, D), dtype=NP_E3M4)
        wmf = np.zeros((T * 128, 16), dtype=np.float32)
        off = 0
        for j, b in enumerate(assign[core]):
            posb = np.nonzero(used[b])[0]
            L = len(posb)
            if L:
                packed[off : off + L] = embs[b, posb].astype(NP_E3M4)
                wmf[off : off + L, 2 * j] = w1[b, posb]
                wmf[off : off + L, 2 * j + 1] = w2[b, posb]
            off += L
        wm16 = np.ascontiguousarray(
            wmf.reshape(T, 128, 16).transpose(1, 0, 2).reshape(128, T * 16)
        ).astype(np.float16)
        in_maps.append(
            {
                "embs": packed,
                "wm": wm16,
                "dw": dw_b,
                "db": db_r,
                "ow": ow_b,
                "ob": ob_r,
                "ones": ones,
            }
        )

    res = run_bass_kernel_spmd(nc, in_maps, list(range(N_CORES)))
    global LAST_RESULTS
    LAST_RESULTS = res

    logits = np.empty((B, 2), np.float32)
    for core in range(N_CORES):
        logits[assign[core]] = res.results[core]["out"].T
    logits[nan_rows] = np.nan
    return logits


# revision 24
# speedup vs baseline: 1.0265x; 1.0025x over previous
"""Trainium2 Bass kernel for BilingualSentenceClassifier (segment_reduce).

Computes, for B=64 samples of S=2048 tokens with D=1024 embedding dims:
  sent1 = mean(embs[1:idx1])            (idx1 = first EOS position)
  sent2 = mean(embs[idx1+2:idx2])       (idx2 = first PAD position - 1)
  logits = tanh(concat(sent1, sent2) @ dense_w + dense_b) @ out_w + out_b

Strategy: pure data parallel over 8 NeuronCores (8 samples per core).
The kernel is HBM-bandwidth bound, so the embedding stream is cut to the
minimum: only tokens that carry nonzero mask weight are shipped, packed
back-to-back at token granularity (samples balanced across cores by exact
token count), and quantized to fp8 e3m4 (the segment means + dense head
keep ~9e-3 relative error, well under the 2e-2 gate).  dense_w streams in
fp8 e3m4 too, pre-scaled by an exact power of two that the tanh
activation's scale input undoes.  Mask weights (1/n at member tokens)
ride in fp16 as the matmul moving operand, so the per-sample
normalization is exact to fp16.

Phase 1 uses the embedding chunk as the *stationary* operand ([128 tok,
128 dims] slices) against the [128 tok, 16] weight matrix, producing the
segment means directly transposed ([dim, 2*sample]) in a single PSUM
bank, which feeds the dense head with no transpose stage: the head runs
with dense_w blocks stationary and [128, 8] moving slices, dense_w
streaming *after* the embeddings so the head chases the tail of the DMA
stream.  Everything downstream of the segment sums stays in fp16/fp32.
"""

import sys

sys.path.insert(0, "/opt/trn_rl_repo")

import numpy as np
import ml_dtypes

import concourse.bass as bass
import concourse.tile as tile
from concourse import mybir
import bass_rust
from concourse.bass_utils import run_bass_kernel_spmd

B, S, D = 64, 2048, 1024
EOS_ID, PAD_ID = 2, 1
N_CORES = 8
B_LOC = B // N_CORES          # samples per core
KD = 16                       # 128-row contraction blocks in dense_w
KH = D // 128                 # 128-row contraction blocks in out_w
G = 16                        # sequence chunks per embedding DMA
ALPHA = 128.0                 # dense_w fp8 pre-scale (exact power of two)

F32 = mybir.dt.float32
F32R = mybir.dt.float32r
BF16 = mybir.dt.bfloat16
F16 = mybir.dt.float16
F8E3 = mybir.dt.float8e3

NP_E3M4 = ml_dtypes.float8_e3m4
NP_BF16 = ml_dtypes.bfloat16


def _split_excess_waits(nc, max_waits=1):
    """This container's walrus rejects instructions carrying more than 1-2
    sync waits (e.g. the Tile tail drain).  Hoist excess waits onto
    preceding same-engine NOPs — semantically identical: the engine's
    sequencer blocks on the NOP's wait before dispatching the original
    instruction."""
    cnt = 0
    for f in nc.m.functions:
        for blk in f.blocks:
            out = []
            changed = False
            for inst in blk.instructions:
                si = inst.sync_info
                if si is not None and len(si.on_wait) > max_waits:
                    waits = list(si.on_wait)
                    for w in waits[:-max_waits]:
                        cnt += 1
                        nop = mybir.InstNoOp(name=f"{inst.name}-hw{cnt}")
                        nop.engine = inst.engine
                        nop.sync_info = bass_rust.SyncInfo(on_wait=[w], on_update=[])
                        out.append(nop)
                    inst.sync_info = bass_rust.SyncInfo(
                        on_wait=waits[-max_waits:], on_update=list(si.on_update)
                    )
                    changed = True
                out.append(inst)
            if changed:
                blk.instructions = out
    return cnt


def _build_program_untrimmed(T, rows_last):
    nc = _build_program(T, rows_last, trim=False)
    return nc


def _build_program(T, rows_last, trim=True):
    """SPMD program processing T 128-token chunks of packed embeddings; the
    final chunk only carries rows_last valid token rows."""
    nc = bass.Bass("TRN2", target_bir_lowering=False, debug=False, num_devices=N_CORES)

    embs = nc.dram_tensor("embs", [T * 128, D], F8E3, kind="ExternalInput")
    wm = nc.dram_tensor("wm", [128, T * 16], F16, kind="ExternalInput")
    dw = nc.dram_tensor("dw", [2 * D, D], F8E3, kind="ExternalInput")
    db = nc.dram_tensor("db", [1, D], F32R, kind="ExternalInput")
    # ow pre-packed on host to [128, KH*2] (partition-major) so the DMA
    # moves one 32B run per partition instead of 2048 4-byte scatters
    ow = nc.dram_tensor("ow", [128, KH * 2], BF16, kind="ExternalInput")
    ob = nc.dram_tensor("ob", [1, 2], F32R, kind="ExternalInput")
    ones = nc.dram_tensor("ones", [1, B_LOC], F32R, kind="ExternalInput")
    out = nc.dram_tensor("out", [2, B_LOC], F32, kind="ExternalOutput")

    groups = []
    t0 = 0
    while t0 < T:
        groups.append((t0, min(G, T - t0)))
        t0 += min(G, T - t0)

    with tile.TileContext(nc) as tc:
        with (
            tc.tile_pool(name="sb", bufs=1) as consts,
            tc.tile_pool(name="ps", bufs=1, space="PSUM") as ps,
        ):
            embp = dwp = consts
            # ---- phase 1: segment sums, directly transposed ---------------
            # xt_ps[p, s, q] = sum_tok emb[tok, 128 s + p] * wm[tok, q]
            # (q = 2 j + r selects sample j / segment r; wm carries 1/n).
            # All 8 dim-slices accumulate into one PSUM bank: start=True only
            # on the very first matmul (clears the bank's has_written bits);
            # every later first-touch overwrites-where-unset, then
            # accumulates.
            # The first embedding group's DMA is issued before the params so
            # the param DMAs' issue overhead hides under its transfer.
            xt_ps = ps.tile([128, 8, 16], F32, tag="xt_ps")
            wm_t = consts.tile([128, T, 16], F16, tag="wm")
            ow_t = consts.tile([128, KH, 2], BF16, tag="ow")
            db_t = consts.tile([1, D], F32R, tag="db")
            ob_t = consts.tile([1, 2], F32R, tag="ob")
            ones_t = consts.tile([1, B_LOC], F32R, tag="ones")
            warm = consts.tile([1, 8], F32, tag="warm")
            for g, (gt, gn) in enumerate(groups):
                et = embp.tile([128, gn, D], F8E3, tag=f"emb{g}")
                nfull = gn if (gt + gn < T or rows_last == 128) else gn - 1
                if nfull:
                    src = embs.ap()[gt * 128 : (gt + nfull) * 128, :]
                    nc.sync.dma_start(
                        out=et[:, :nfull, :],
                        in_=src.rearrange("(n p) d -> p n d", p=128),
                    )
                if nfull < gn:
                    base = (gt + nfull) * 128
                    nc.sync.dma_start(
                        out=et[:rows_last, nfull, :],
                        in_=embs.ap()[base : base + rows_last, :],
                    )
                if g == 0:
                    nc.sync.dma_start(out=wm_t[:], in_=wm.ap())
                    nc.sync.dma_start(out=db_t[:], in_=db.ap())
                    nc.sync.dma_start(out=ones_t[:], in_=ones.ap())
                    # warm the ScalarE Tanh LUT while the stream runs
                    nc.vector.memset(warm[:], 0.0)
                    nc.scalar.activation(
                        warm[:], warm[:], mybir.ActivationFunctionType.Tanh
                    )
                for c in range(gn):
                    t = gt + c
                    rows = 128 if t < T - 1 else rows_last
                    for s in range(8):
                        nc.tensor.matmul(
                            xt_ps[:, s, :],
                            et[0:rows, c, s * 128 : s * 128 + 128],
                            wm_t[0:rows, t, :],
                            start=(t == 0 and s == 0),
                            stop=(t == T - 1),
                        )
            xt = consts.tile([128, 8, 16], F16, tag="xt")
            nc.vector.tensor_copy(xt[:], xt_ps[:])

            # dense_w streams after the embeddings; the head chases it.
            # Batched into >=3-block DMAs (transfer > the 625ns issue cost)
            # with a small final DMA so the post-stream matmul tail is short.
            dw_t = dwp.tile([128, KD, D], F8E3, tag="dw")
            k0 = 0
            for nblk in (3, 3, 3, 3, 3, 1):
                src = dw.ap()[128 * k0 : 128 * (k0 + nblk), :]
                nc.sync.dma_start(
                    out=dw_t[:, k0 : k0 + nblk, :],
                    in_=src.rearrange("(n p) d -> p n d", p=128),
                )
                k0 += nblk
            # ow/ob stream last: their consumers (the logits head) are gated
            # by the tanh chain anyway, so their completion latency hides
            # under it, and dense_w's completion semaphore starts earlier
            nc.sync.dma_start(out=ow_t[:], in_=ow.ap())
            nc.sync.dma_start(out=ob_t[:], in_=ob.ap())

            # ---- phase 2: hidden^T = tanh(dense_w^T x + db), k-major ------
            # The db bias matmuls lead the group (start=True on the first
            # clears the bank) so nothing but the final k-block's 8 matmuls
            # remains after the last dense_w DMA lands.
            ph = ps.tile([128, KH, B_LOC], F32, tag="ph")
            for h in range(KH):
                nc.tensor.matmul(
                    ph[:, h, :],
                    db_t[0:1, h * 128 : h * 128 + 128],
                    ones_t[0:1, :],
                    start=(h == 0),
                    stop=False,
                )
            for k in range(KD):
                r, s = divmod(k, 8)
                mov = xt[:, s, r::2]
                for h in range(KH):
                    nc.tensor.matmul(
                        ph[:, h, :],
                        dw_t[:, k, h * 128 : h * 128 + 128],
                        mov,
                        start=False,
                        stop=(k == KD - 1),
                    )
            # ph holds ALPHA*(x @ dense_w + db); the activation's exact
            # power-of-two scale undoes the fp8 weight pre-scale
            ht = consts.tile([128, KH, B_LOC], F16, tag="ht")
            nc.scalar.activation(
                ht[:], ph[:], mybir.ActivationFunctionType.Tanh, scale=1.0 / ALPHA
            )

            # ---- phase 3: logits^T = out_w^T h + ob -----------------------
            pl = ps.tile([2, B_LOC], F32, tag="pl")
            for h in range(KH):
                nc.tensor.matmul(
                    pl[:], ow_t[:, h, :], ht[:, h, :], start=(h == 0), stop=False
                )
            nc.tensor.matmul(
                pl[:], ob_t[0:1, :], ones_t[0:1, :], start=False, stop=True
            )
            lg = consts.tile([2, B_LOC], F32, tag="lg")
            nc.vector.tensor_copy(lg[:], pl[:])
            nc.sync.dma_start(out=out.ap(), in_=lg[:])

    _split_excess_waits(nc)
    if trim:
        try:
            _trim_framework_sync(nc, do_prologue=True, do_epilogue=True)
        except Exception:
            # the trim is a pure optimization; an unexpected program shape
            # must not break the build (rebuild untrimmed)
            return _build_program(T, rows_last, trim=False)
    return nc


def _trim_framework_sync(nc, do_prologue=True, do_epilogue=True):
    """Post-pass on the Tile-generated program:
    1. Remove the prologue all-engine barrier — every body dependency is
       already semaphore-protected, and each engine's own setup precedes
       its body in program order.
    2. Drop the epilogue drain's waits on DMA-lane semaphores that were
       already consumed by in-body readers; only the final (output) DMA's
       lane has no in-body consumer, so only its wait is load-bearing.
    The epilogue barrier + semaphore clears are kept so repeat launches
    still start from clean semaphore state."""
    f = nc.m.functions[0]
    pre, body, epi = f.blocks[0], f.blocks[1], f.blocks[-1]

    removed_update_sems = set()
    out = []
    for inst in pre.instructions if do_prologue else []:
        if isinstance(inst, mybir.InstRegisterMove):
            # zero/broadcast register init; nothing in this program reads
            # registers (all access patterns are static)
            continue
        if isinstance(inst, mybir.InstEventSemaphore) and inst.name.startswith(
            "barrier_"
        ):
            if inst.sync_info:
                removed_update_sems.update(u.id for u in inst.sync_info.on_update)
            continue
        if isinstance(inst, mybir.InstDrain):
            # the prologue drains only existed to serve the barrier
            continue
        out.append(inst)
    if do_prologue:
        pre.instructions = out

    # epilogue: rebuild it minimally.  The only load-bearing wait is the
    # output DMA's lane semaphore (every other DMA's semaphore was already
    # consumed by an in-body reader, and the out DMA transitively follows
    # all compute), so the epilogue becomes: SP drain waiting that
    # semaphore, the semaphore RANGE_CLEAR on SP right after it (clean
    # state for repeat launches), and one sync-free pipeline drain per
    # other engine.  Both Tile all-engine barrier rounds are dead weight.
    if do_epilogue:
        dmas = [i for i in body.instructions if isinstance(i, mybir.InstDMACopy)]
        out_upd = dmas[-1].sync_info.on_update[0]
        out_sem = out_upd.id
        final_val = sum(
            int(u.update_value or 0)
            for d in dmas
            for u in (d.sync_info.on_update if d.sync_info else [])
            if u.id == out_sem
        )
        out_wait = None
        for inst in epi.instructions:
            for w in inst.sync_info.on_wait if inst.sync_info else []:
                if w.id == out_sem:
                    out_wait = w
        assert out_wait is not None and int(out_wait.wait_value) == final_val, (
            f"epilogue lacks a wait for the out DMA sem {out_sem} at {final_val}"
        )
        sp_drain = None
        clear_inst = None
        eng_drains = {}
        for inst in epi.instructions:
            tn = type(inst).__name__
            if tn == "InstISA" and clear_inst is None:
                clear_inst = inst
            if tn == "InstDrain":
                key = str(inst.engine)
                if "SP" in key and sp_drain is None:
                    sp_drain = inst
                elif "SP" not in key:
                    eng_drains.setdefault(key, inst)
        assert sp_drain is not None and clear_inst is not None
        sp_drain.sync_info = bass_rust.SyncInfo(on_wait=[out_wait], on_update=[])
        clear_inst.engine = sp_drain.engine
        rebuilt = [sp_drain, clear_inst]
        for inst in eng_drains.values():
            inst.sync_info = None
            rebuilt.append(inst)
        epi.instructions = rebuilt

    # hoist the first SP DMA issue ahead of SP's block-0 branch so the
    # HWDGE stage starts ~50ns earlier (the branch runs during the DGE
    # pipeline delay instead of before the issue)
    if do_prologue:
        first_dma = next(
            (
                i
                for i, inst in enumerate(body.instructions)
                if isinstance(inst, mybir.InstDMACopy)
                and not (inst.sync_info and inst.sync_info.on_wait)
            ),
            None,
        )
        sp_branch = next(
            (
                i
                for i, inst in enumerate(pre.instructions)
                if type(inst).__name__ == "InstUnconditionalBranch"
                and str(inst.engine) == str(body.instructions[first_dma].engine)
            ),
            None,
        ) if first_dma is not None else None
        if first_dma is not None and sp_branch is not None:
            dma_inst = body.instructions.pop(first_dma)
            pre.instructions.insert(sp_branch, dma_inst)

    # sanity: every waited (sem, value) must be coverable by total updates,
    # and nothing may wait on a semaphore whose barrier update was removed
    from collections import defaultdict

    updates = defaultdict(int)
    waited = defaultdict(int)
    for blk in f.blocks:
        for inst in blk.instructions:
            si = inst.sync_info
            if not si:
                continue
            for u in si.on_update:
                updates[u.id] += int(u.update_value or 0)
            for w in si.on_wait:
                if w.wait_value is not None and "ge" in str(w.wait_mode):
                    waited[w.id] = max(waited[w.id], int(w.wait_value))
    for sem_id, val in waited.items():
        assert updates[sem_id] >= val, (
            f"sem {sem_id}: waits up to {val} but only {updates[sem_id]} updates"
        )
        assert sem_id not in removed_update_sems or updates[sem_id] >= val


_PROGRAM_CACHE = {}
LAST_RESULTS = None


def kernel(embs, input_ids, dense_w, dense_b, out_w, out_b):
    embs = np.ascontiguousarray(np.asarray(embs, dtype=np.float32))
    ids = np.asarray(input_ids)
    dense_w = np.asarray(dense_w, dtype=np.float32)
    dense_b = np.asarray(dense_b, dtype=np.float32)
    out_w = np.asarray(out_w, dtype=np.float32)
    out_b = np.asarray(out_b, dtype=np.float32)

    # host-side mask metadata — exactly the reference's argmax semantics
    idx1 = np.argmax(ids == EOS_ID, axis=-1)
    idx2 = np.argmax(ids == PAD_ID, axis=-1) - 1
    pos = np.arange(S)
    m1 = ((pos >= 1) & (pos < idx1[:, None])).astype(np.float32)
    m2 = ((pos >= idx1[:, None] + 2) & (pos < idx2[:, None])).astype(np.float32)
    n1 = m1.sum(-1, keepdims=True)
    n2 = m2.sum(-1, keepdims=True)
    # empty segments give 0/0 = NaN in the reference; keep device weights
    # finite (zero) and reinstate the NaN on the host afterwards
    w1 = np.where(n1 > 0, m1 / np.maximum(n1, 1), 0.0).astype(np.float32)
    w2 = np.where(n2 > 0, m2 / np.maximum(n2, 1), 0.0).astype(np.float32)
    nan_rows = (n1[:, 0] == 0) | (n2[:, 0] == 0)

    used = (m1 + m2) > 0                      # [B, S] tokens with weight
    tokens = used.sum(axis=1).astype(int)

    # balance samples across cores by exact token count (8 per core)
    order = np.argsort(-tokens, kind="stable")
    loads = np.zeros(N_CORES, dtype=int)
    counts = np.zeros(N_CORES, dtype=int)
    assign = [[] for _ in range(N_CORES)]
    for b in order:
        free = np.nonzero(counts < B_LOC)[0]
        core = free[np.argmin(loads[free])]
        assign[core].append(int(b))
        loads[core] += tokens[b]
        counts[core] += 1
    # pairwise-swap refinement: the exactly-8-per-core constraint leaves the
    # greedy pass a few hundred tokens imbalanced, which costs whole
    # 128-token chunks of stream time on every core
    for _ in range(200):
        i = int(np.argmax(loads))
        best = None
        for j in range(N_CORES):
            if j == i:
                continue
            for ai, a in enumerate(assign[i]):
                for bj, b in enumerate(assign[j]):
                    d = tokens[a] - tokens[b]
                    if d <= 0:
                        continue
                    new_max = max(loads[i] - d, loads[j] + d)
                    if new_max < loads[i] and (best is None or new_max < best[0]):
                        best = (new_max, j, ai, bj)
        if best is None:
            break
        _, j, ai, bj = best
        a, b = assign[i][ai], assign[j][bj]
        assign[i][ai], assign[j][bj] = b, a
        loads[i] += tokens[b] - tokens[a]
        loads[j] += tokens[a] - tokens[b]
    T = max(1, int(-(-loads.max() // 128)))
    rows_last = max(1, int(loads.max() - (T - 1) * 128))

    key = (T, rows_last)
    if key not in _PROGRAM_CACHE:
        _PROGRAM_CACHE[key] = _build_program(T, rows_last)
    nc = _PROGRAM_CACHE[key]

    dw_b = (dense_w * ALPHA).astype(NP_E3M4)
    ow_b = np.ascontiguousarray(
        out_w.reshape(KH, 128, 2).transpose(1, 0, 2).reshape(128, KH * 2)
    ).astype(NP_BF16)
    db_r = (dense_b * ALPHA).reshape(1, D)
    ob_r = out_b.reshape(1, 2)
    ones = np.ones((1, B_LOC), np.float32)

    in_maps = []
    for core in range(N_CORES):
        packed = np.zeros((T * 128, D), dtype=NP_E3M4)
        wmf = np.zeros((T * 128, 16), dtype=np.float32)
        off = 0
        for j, b in enumerate(assign[core]):
            posb = np.nonzero(used[b])[0]
            L = len(posb)
            if L:
                packed[off : off + L] = embs[b, posb].astype(NP_E3M4)
                wmf[off : off + L, 2 * j] = w1[b, posb]
                wmf[off : off + L, 2 * j + 1] = w2[b, posb]
            off += L
        wm16 = np.ascontiguousarray(
            wmf.reshape(T, 128, 16).transpose(1, 0, 2).reshape(128, T * 16)
        ).astype(np.float16)
        in_maps.append(
            {
                "embs": packed,
                "wm": wm16,
                "dw": dw_b,
                "db": db_r,
                "ow": ow_b,
                "ob": ob_r,
                "ones": ones,
            }
        )

    res = run_bass_kernel_spmd(nc, in_maps, list(range(N_CORES)))
    global LAST_RESULTS
    LAST_RESULTS = res

    logits = np.empty((B, 2), np.float32)
    for core in range(N_CORES):
        logits[assign[core]] = res.results[core]["out"].T
    logits[nan_rows] = np.nan
    return logits


# revision 25
# speedup vs baseline: 1.0269x; 1.0004x over previous
"""Trainium2 Bass kernel for BilingualSentenceClassifier (segment_reduce).

Computes, for B=64 samples of S=2048 tokens with D=1024 embedding dims:
  sent1 = mean(embs[1:idx1])            (idx1 = first EOS position)
  sent2 = mean(embs[idx1+2:idx2])       (idx2 = first PAD position - 1)
  logits = tanh(concat(sent1, sent2) @ dense_w + dense_b) @ out_w + out_b

Strategy: pure data parallel over 8 NeuronCores (8 samples per core).
The kernel is HBM-bandwidth bound, so the embedding stream is cut to the
minimum: only tokens that carry nonzero mask weight are shipped, packed
back-to-back at token granularity (samples balanced across cores by exact
token count), and quantized to fp8 e3m4 (the segment means + dense head
keep ~9e-3 relative error, well under the 2e-2 gate).  dense_w streams in
fp8 e3m4 too, pre-scaled by an exact power of two that the tanh
activation's scale input undoes.  Mask weights (1/n at member tokens)
ride in fp16 as the matmul moving operand, so the per-sample
normalization is exact to fp16.

Phase 1 uses the embedding chunk as the *stationary* operand ([128 tok,
128 dims] slices) against the [128 tok, 16] weight matrix, producing the
segment means directly transposed ([dim, 2*sample]) in a single PSUM
bank, which feeds the dense head with no transpose stage: the head runs
with dense_w blocks stationary and [128, 8] moving slices, dense_w
streaming *after* the embeddings so the head chases the tail of the DMA
stream.  Everything downstream of the segment sums stays in fp16/fp32.
"""

import sys

sys.path.insert(0, "/opt/trn_rl_repo")

import numpy as np
import ml_dtypes

import concourse.bass as bass
import concourse.tile as tile
from concourse import mybir
import bass_rust
from concourse.bass_utils import run_bass_kernel_spmd

B, S, D = 64, 2048, 1024
EOS_ID, PAD_ID = 2, 1
N_CORES = 8
B_LOC = B // N_CORES          # samples per core
KD = 16                       # 128-row contraction blocks in dense_w
KH = D // 128                 # 128-row contraction blocks in out_w
G = 16                        # sequence chunks per embedding DMA
ALPHA = 128.0                 # dense_w fp8 pre-scale (exact power of two)

F32 = mybir.dt.float32
F32R = mybir.dt.float32r
BF16 = mybir.dt.bfloat16
F16 = mybir.dt.float16
F8E3 = mybir.dt.float8e3

NP_E3M4 = ml_dtypes.float8_e3m4
NP_BF16 = ml_dtypes.bfloat16


def _split_excess_waits(nc, max_waits=1):
    """This container's walrus rejects instructions carrying more than 1-2
    sync waits (e.g. the Tile tail drain).  Hoist excess waits onto
    preceding same-engine NOPs — semantically identical: the engine's
    sequencer blocks on the NOP's wait before dispatching the original
    instruction."""
    cnt = 0
    for f in nc.m.functions:
        for blk in f.blocks:
            out = []
            changed = False
            for inst in blk.instructions:
                si = inst.sync_info
                if si is not None and len(si.on_wait) > max_waits:
                    waits = list(si.on_wait)
                    for w in waits[:-max_waits]:
                        cnt += 1
                        nop = mybir.InstNoOp(name=f"{inst.name}-hw{cnt}")
                        nop.engine = inst.engine
                        nop.sync_info = bass_rust.SyncInfo(on_wait=[w], on_update=[])
                        out.append(nop)
                    inst.sync_info = bass_rust.SyncInfo(
                        on_wait=waits[-max_waits:], on_update=list(si.on_update)
                    )
                    changed = True
                out.append(inst)
            if changed:
                blk.instructions = out
    return cnt


def _build_program_untrimmed(T, rows_last):
    nc = _build_program(T, rows_last, trim=False)
    return nc


def _build_program(T, rows_last, trim=True):
    """SPMD program processing T 128-token chunks of packed embeddings; the
    final chunk only carries rows_last valid token rows."""
    nc = bass.Bass("TRN2", target_bir_lowering=False, debug=False, num_devices=N_CORES)

    embs = nc.dram_tensor("embs", [T * 128, D], F8E3, kind="ExternalInput")
    wm = nc.dram_tensor("wm", [128, T * 16], F16, kind="ExternalInput")
    dw = nc.dram_tensor("dw", [2 * D, D], F8E3, kind="ExternalInput")
    db = nc.dram_tensor("db", [1, D], F32R, kind="ExternalInput")
    # ow pre-packed on host to [128, KH*2] (partition-major) so the DMA
    # moves one 32B run per partition instead of 2048 4-byte scatters
    ow = nc.dram_tensor("ow", [128, KH * 2], BF16, kind="ExternalInput")
    ob = nc.dram_tensor("ob", [1, 2], F32R, kind="ExternalInput")
    ones = nc.dram_tensor("ones", [1, B_LOC], F32R, kind="ExternalInput")
    out = nc.dram_tensor("out", [2, B_LOC], F32, kind="ExternalOutput")

    groups = []
    t0 = 0
    while t0 < T:
        groups.append((t0, min(G, T - t0)))
        t0 += min(G, T - t0)

    with tile.TileContext(nc) as tc:
        with (
            tc.tile_pool(name="sb", bufs=1) as consts,
            tc.tile_pool(name="ps", bufs=1, space="PSUM") as ps,
        ):
            embp = dwp = consts
            # ---- phase 1: segment sums, directly transposed ---------------
            # xt_ps[p, s, q] = sum_tok emb[tok, 128 s + p] * wm[tok, q]
            # (q = 2 j + r selects sample j / segment r; wm carries 1/n).
            # All 8 dim-slices accumulate into one PSUM bank: start=True only
            # on the very first matmul (clears the bank's has_written bits);
            # every later first-touch overwrites-where-unset, then
            # accumulates.
            # The first embedding group's DMA is issued before the params so
            # the param DMAs' issue overhead hides under its transfer.
            xt_ps = ps.tile([128, 8, 16], F32, tag="xt_ps")
            wm_t = consts.tile([128, T, 16], F16, tag="wm")
            ow_t = consts.tile([128, KH, 2], BF16, tag="ow")
            db_t = consts.tile([1, D], F32R, tag="db")
            ob_t = consts.tile([1, 2], F32R, tag="ob")
            ones_t = consts.tile([1, B_LOC], F32R, tag="ones")
            warm = consts.tile([1, 8], F32, tag="warm")
            for g, (gt, gn) in enumerate(groups):
                et = embp.tile([128, gn, D], F8E3, tag=f"emb{g}")
                nfull = gn if (gt + gn < T or rows_last == 128) else gn - 1
                if nfull:
                    src = embs.ap()[gt * 128 : (gt + nfull) * 128, :]
                    nc.sync.dma_start(
                        out=et[:, :nfull, :],
                        in_=src.rearrange("(n p) d -> p n d", p=128),
                    )
                if nfull < gn:
                    base = (gt + nfull) * 128
                    nc.sync.dma_start(
                        out=et[:rows_last, nfull, :],
                        in_=embs.ap()[base : base + rows_last, :],
                    )
                if g == 0:
                    nc.sync.dma_start(out=wm_t[:], in_=wm.ap())
                    nc.sync.dma_start(out=db_t[:], in_=db.ap())
                    nc.sync.dma_start(out=ones_t[:], in_=ones.ap())
                    # warm the ScalarE Tanh LUT while the stream runs
                    nc.vector.memset(warm[:], 0.0)
                    nc.scalar.activation(
                        warm[:], warm[:], mybir.ActivationFunctionType.Tanh
                    )
                for c in range(gn):
                    t = gt + c
                    rows = 128 if t < T - 1 else rows_last
                    for s in range(8):
                        nc.tensor.matmul(
                            xt_ps[:, s, :],
                            et[0:rows, c, s * 128 : s * 128 + 128],
                            wm_t[0:rows, t, :],
                            start=(t == 0 and s == 0),
                            stop=(t == T - 1),
                        )
            xt = consts.tile([128, 8, 16], F16, tag="xt")
            nc.vector.tensor_copy(xt[:], xt_ps[:])

            # dense_w streams after the embeddings; the head chases it.
            # Batched into >=3-block DMAs (transfer > the 625ns issue cost)
            # with a small final DMA so the post-stream matmul tail is short.
            dw_t = dwp.tile([128, KD, D], F8E3, tag="dw")
            k0 = 0
            for nblk in (3, 3, 3, 3, 3):
                src = dw.ap()[128 * k0 : 128 * (k0 + nblk), :]
                nc.sync.dma_start(
                    out=dw_t[:, k0 : k0 + nblk, :],
                    in_=src.rearrange("(n p) d -> p n d", p=128),
                )
                k0 += nblk
            # the final k-block ships as two 512-col halves (512B runs stay
            # at full descriptor rate) so only 4 of its 8 matmuls wait on
            # the very last completion semaphore
            for half in range(2):
                cols = slice(half * (D // 2), (half + 1) * (D // 2))
                nc.sync.dma_start(
                    out=dw_t[:, KD - 1, cols],
                    in_=dw.ap()[128 * (KD - 1) : 128 * KD, cols],
                )
            # ow/ob stream last: their consumers (the logits head) are gated
            # by the tanh chain anyway, so their completion latency hides
            # under it, and dense_w's completion semaphore starts earlier
            nc.sync.dma_start(out=ow_t[:], in_=ow.ap())
            nc.sync.dma_start(out=ob_t[:], in_=ob.ap())

            # ---- phase 2: hidden^T = tanh(dense_w^T x + db), k-major ------
            # The db bias matmuls lead the group (start=True on the first
            # clears the bank) so nothing but the final k-block's 8 matmuls
            # remains after the last dense_w DMA lands.
            ph = ps.tile([128, KH, B_LOC], F32, tag="ph")
            for h in range(KH):
                nc.tensor.matmul(
                    ph[:, h, :],
                    db_t[0:1, h * 128 : h * 128 + 128],
                    ones_t[0:1, :],
                    start=(h == 0),
                    stop=False,
                )
            for k in range(KD):
                r, s = divmod(k, 8)
                mov = xt[:, s, r::2]
                for h in range(KH):
                    nc.tensor.matmul(
                        ph[:, h, :],
                        dw_t[:, k, h * 128 : h * 128 + 128],
                        mov,
                        start=False,
                        stop=(k == KD - 1),
                    )
            # ph holds ALPHA*(x @ dense_w + db); the activation's exact
            # power-of-two scale undoes the fp8 weight pre-scale
            ht = consts.tile([128, KH, B_LOC], F16, tag="ht")
            nc.scalar.activation(
                ht[:], ph[:], mybir.ActivationFunctionType.Tanh, scale=1.0 / ALPHA
            )

            # ---- phase 3: logits^T = out_w^T h + ob -----------------------
            pl = ps.tile([2, B_LOC], F32, tag="pl")
            nc.tensor.matmul(
                pl[:], ob_t[0:1, :], ones_t[0:1, :], start=True, stop=False
            )
            for h in range(KH):
                nc.tensor.matmul(
                    pl[:], ow_t[:, h, :], ht[:, h, :], start=False, stop=(h == KH - 1)
                )
            lg = consts.tile([2, B_LOC], F32, tag="lg")
            nc.vector.tensor_copy(lg[:], pl[:])
            nc.sync.dma_start(out=out.ap(), in_=lg[:])

    _split_excess_waits(nc)
    if trim:
        try:
            _trim_framework_sync(nc, do_prologue=True, do_epilogue=True)
        except Exception:
            # the trim is a pure optimization; an unexpected program shape
            # must not break the build (rebuild untrimmed)
            return _build_program(T, rows_last, trim=False)
    return nc


def _trim_framework_sync(nc, do_prologue=True, do_epilogue=True):
    """Post-pass on the Tile-generated program:
    1. Remove the prologue all-engine barrier — every body dependency is
       already semaphore-protected, and each engine's own setup precedes
       its body in program order.
    2. Drop the epilogue drain's waits on DMA-lane semaphores that were
       already consumed by in-body readers; only the final (output) DMA's
       lane has no in-body consumer, so only its wait is load-bearing.
    The epilogue barrier + semaphore clears are kept so repeat launches
    still start from clean semaphore state."""
    f = nc.m.functions[0]
    pre, body, epi = f.blocks[0], f.blocks[1], f.blocks[-1]

    removed_update_sems = set()
    out = []
    for inst in pre.instructions if do_prologue else []:
        if isinstance(inst, mybir.InstRegisterMove):
            # zero/broadcast register init; nothing in this program reads
            # registers (all access patterns are static)
            continue
        if isinstance(inst, mybir.InstEventSemaphore) and inst.name.startswith(
            "barrier_"
        ):
            if inst.sync_info:
                removed_update_sems.update(u.id for u in inst.sync_info.on_update)
            continue
        if isinstance(inst, mybir.InstDrain):
            # the prologue drains only existed to serve the barrier
            continue
        out.append(inst)
    if do_prologue:
        pre.instructions = out

    # epilogue: rebuild it minimally.  The only load-bearing wait is the
    # output DMA's lane semaphore (every other DMA's semaphore was already
    # consumed by an in-body reader, and the out DMA transitively follows
    # all compute), so the epilogue becomes: SP drain waiting that
    # semaphore, the semaphore RANGE_CLEAR on SP right after it (clean
    # state for repeat launches), and one sync-free pipeline drain per
    # other engine.  Both Tile all-engine barrier rounds are dead weight.
    if do_epilogue:
        dmas = [i for i in body.instructions if isinstance(i, mybir.InstDMACopy)]
        out_upd = dmas[-1].sync_info.on_update[0]
        out_sem = out_upd.id
        final_val = sum(
            int(u.update_value or 0)
            for d in dmas
            for u in (d.sync_info.on_update if d.sync_info else [])
            if u.id == out_sem
        )
        out_wait = None
        for inst in epi.instructions:
            for w in inst.sync_info.on_wait if inst.sync_info else []:
                if w.id == out_sem:
                    out_wait = w
        assert out_wait is not None and int(out_wait.wait_value) == final_val, (
            f"epilogue lacks a wait for the out DMA sem {out_sem} at {final_val}"
        )
        sp_drain = None
        clear_inst = None
        eng_drains = {}
        for inst in epi.instructions:
            tn = type(inst).__name__
            if tn == "InstISA" and clear_inst is None:
                clear_inst = inst
            if tn == "InstDrain":
                key = str(inst.engine)
                if "SP" in key and sp_drain is None:
                    sp_drain = inst
                elif "SP" not in key:
                    eng_drains.setdefault(key, inst)
        assert sp_drain is not None and clear_inst is not None
        sp_drain.sync_info = bass_rust.SyncInfo(on_wait=[out_wait], on_update=[])
        clear_inst.engine = sp_drain.engine
        rebuilt = [sp_drain, clear_inst]
        for inst in eng_drains.values():
            inst.sync_info = None
            rebuilt.append(inst)
        epi.instructions = rebuilt

    # hoist the first SP DMA issue ahead of SP's block-0 branch so the
    # HWDGE stage starts ~50ns earlier (the branch runs during the DGE
    # pipeline delay instead of before the issue)
    if do_prologue:
        first_dma = next(
            (
                i
                for i, inst in enumerate(body.instructions)
                if isinstance(inst, mybir.InstDMACopy)
                and not (inst.sync_info and inst.sync_info.on_wait)
            ),
            None,
        )
        sp_branch = next(
            (
                i
                for i, inst in enumerate(pre.instructions)
                if type(inst).__name__ == "InstUnconditionalBranch"
                and str(inst.engine) == str(body.instructions[first_dma].engine)
            ),
            None,
        ) if first_dma is not None else None
        if first_dma is not None and sp_branch is not None:
            dma_inst = body.instructions.pop(first_dma)
            pre.instructions.insert(sp_branch, dma_inst)

    # sanity: every waited (sem, value) must be coverable by total updates,
    # and nothing may wait on a semaphore whose barrier update was removed
    from collections import defaultdict

    updates = defaultdict(int)
    waited = defaultdict(int)
    for blk in f.blocks:
        for inst in blk.instructions:
            si = inst.sync_info
            if not si:
                continue
            for u in si.on_update:
                updates[u.id] += int(u.update_value or 0)
            for w in si.on_wait:
                if w.wait_value is not None and "ge" in str(w.wait_mode):
                    waited[w.id] = max(waited[w.id], int(w.wait_value))
    for sem_id, val in waited.items():
        assert updates[sem_id] >= val, (
            f"sem {sem_id}: waits up to {val} but only {updates[sem_id]} updates"
        )
        assert sem_id not in removed_update_sems or updates[sem_id] >= val


_PROGRAM_CACHE = {}
LAST_RESULTS = None


def kernel(embs, input_ids, dense_w, dense_b, out_w, out_b):
    embs = np.ascontiguousarray(np.asarray(embs, dtype=np.float32))
    ids = np.asarray(input_ids)
    dense_w = np.asarray(dense_w, dtype=np.float32)
    dense_b = np.asarray(dense_b, dtype=np.float32)
    out_w = np.asarray(out_w, dtype=np.float32)
    out_b = np.asarray(out_b, dtype=np.float32)

    # host-side mask metadata — exactly the reference's argmax semantics
    idx1 = np.argmax(ids == EOS_ID, axis=-1)
    idx2 = np.argmax(ids == PAD_ID, axis=-1) - 1
    pos = np.arange(S)
    m1 = ((pos >= 1) & (pos < idx1[:, None])).astype(np.float32)
    m2 = ((pos >= idx1[:, None] + 2) & (pos < idx2[:, None])).astype(np.float32)
    n1 = m1.sum(-1, keepdims=True)
    n2 = m2.sum(-1, keepdims=True)
    # empty segments give 0/0 = NaN in the reference; keep device weights
    # finite (zero) and reinstate the NaN on the host afterwards
    w1 = np.where(n1 > 0, m1 / np.maximum(n1, 1), 0.0).astype(np.float32)
    w2 = np.where(n2 > 0, m2 / np.maximum(n2, 1), 0.0).astype(np.float32)
    nan_rows = (n1[:, 0] == 0) | (n2[:, 0] == 0)

    used = (m1 + m2) > 0                      # [B, S] tokens with weight
    tokens = used.sum(axis=1).astype(int)

    # balance samples across cores by exact token count (8 per core)
    order = np.argsort(-tokens, kind="stable")
    loads = np.zeros(N_CORES, dtype=int)
    counts = np.zeros(N_CORES, dtype=int)
    assign = [[] for _ in range(N_CORES)]
    for b in order:
        free = np.nonzero(counts < B_LOC)[0]
        core = free[np.argmin(loads[free])]
        assign[core].append(int(b))
        loads[core] += tokens[b]
        counts[core] += 1
    # pairwise-swap refinement: the exactly-8-per-core constraint leaves the
    # greedy pass a few hundred tokens imbalanced, which costs whole
    # 128-token chunks of stream time on every core
    for _ in range(200):
        i = int(np.argmax(loads))
        best = None
        for j in range(N_CORES):
            if j == i:
                continue
            for ai, a in enumerate(assign[i]):
                for bj, b in enumerate(assign[j]):
                    d = tokens[a] - tokens[b]
                    if d <= 0:
                        continue
                    new_max = max(loads[i] - d, loads[j] + d)
                    if new_max < loads[i] and (best is None or new_max < best[0]):
                        best = (new_max, j, ai, bj)
        if best is None:
            break
        _, j, ai, bj = best
        a, b = assign[i][ai], assign[j][bj]
        assign[i][ai], assign[j][bj] = b, a
        loads[i] += tokens[b] - tokens[a]
        loads[j] += tokens[a] - tokens[b]
    T = max(1, int(-(-loads.max() // 128)))
    rows_last = max(1, int(loads.max() - (T - 1) * 128))

    key = (T, rows_last)
    if key not in _PROGRAM_CACHE:
        _PROGRAM_CACHE[key] = _build_program(T, rows_last)
    nc = _PROGRAM_CACHE[key]

    dw_b = (dense_w * ALPHA).astype(NP_E3M4)
    ow_b = np.ascontiguousarray(
        out_w.reshape(KH, 128, 2).transpose(1, 0, 2).reshape(128, KH * 2)
    ).astype(NP_BF16)
    db_r = (dense_b * ALPHA).reshape(1, D)
    ob_r = out_b.reshape(1, 2)
    ones = np.ones((1, B_LOC), np.float32)

    in_maps = []
    for core in range(N_CORES):
        packed = np.zeros((T * 128, D), dtype=NP_E3M4)
        wmf = np.zeros((T * 128, 16), dtype=np.float32)
        off = 0
        for j, b in enumerate(assign[core]):
            posb = np.nonzero(used[b])[0]
            L = len(posb)
            if L:
                packed[off : off + L] = embs[b, posb].astype(NP_E3M4)
                wmf[off : off + L, 2 * j] = w1[b, posb]
                wmf[off : off + L, 2 * j + 1] = w2[b, posb]
            off += L
        wm16 = np.ascontiguousarray(
            wmf.reshape(T, 128, 16).transpose(1, 0, 2).reshape(128, T * 16)
        ).astype(np.float16)
        in_maps.append(
            {
                "embs": packed,
                "wm": wm16,
                "dw": dw_b,
                "db": db_r,
                "ow": ow_b,
                "ob": ob_r,
                "ones": ones,
            }
        )

    res = run_bass_kernel_spmd(nc, in_maps, list(range(N_CORES)))
    global LAST_RESULTS
    LAST_RESULTS = res

    logits = np.empty((B, 2), np.float32)
    for core in range(N_CORES):
        logits[assign[core]] = res.results[core]["out"].T
    logits[nan_rows] = np.nan
    return logits
